# revision 7
# baseline (speedup 1.0000x reference)
"""Trainium2 Bass kernel for nn_Decoder_76974403879078 — v2 (weight-stationary).

2-layer LSTM decoder, B=256, H=512, T=64 steps, argmax feedback.
Sharding: data-parallel over batch, 8 cores x 32; the sequential time loop
runs locally per core (no collectives).

Design vs the 819us baseline (3439us fp32 original):
  - Weight-stationary matmuls: weights are the PE stationary operand
    (lhsT [K=128, M=128]), h the moving operand [K=128, N=32].  The cost
    model charges out-free-size x cycles/row, so streaming 32 batch cols
    instead of 512 gate cols cuts PE stream time ~4x.
  - fp8 e4m3 DoubleRow everywhere big: gate weights, fc2W, fc3W, the E1
    x-path table, and the h states.  One DoubleRow instruction contracts
    two K=128 tiles at 0.5 cycles/row (8x less PE stream than the
    baseline's bf16 output-stationary scheme).  Host study: full-fp8
    trajectory rel err ~4e-3 vs the 2e-2 tolerance; argmax flips are
    benign near-ties (bf16 itself flips 220/16384 with rel 3.9e-4).
    Weights x8, h x4 dodge fp8 subnormals; the 1/32 unscale folds into
    the ACT gate sigmoid/tanh `scale`.
  - Gate PSUM layout per layer: tiles (g|o) and (i|f), each its own
    bank/zero-region, closed per tile so tanh(g) fires after the go-tile
    x-rounds and sigmoid(i,f) right after the if-tile's.
  - L1 x-path: E1ext table matmul with the onehot packed as K=32
    DoubleRow pairs; dur/bias rows ride pair-slot 1 at partitions 0/1
    (constant after t=0, memset once).  argmax feedback: DVE max ->
    is_equal -> 32x32 StreamTranspose writes pair-slot 0 in place.
  - Bias/const injects (B2, CC2, F3) are bf16 identity matmuls
    (lhsT=rows, rhs=I32) that also open each bank's accumulation group -
    no warm-PSUM hacks.
  - leaky(z) -> two fp8 branches on twin PSUM copies of fc2 (tile dep
    tracking serializes same-tile readers): relu*0.99 on DVE (max+mult
    tensor_scalar) || 0.01*z on ACT (copy w/ scale); fc3 = 4 DoubleRow
    matmuls sharing one W3.
  - c-update: u = sig_i*tanh_g (DVE bf16 2x), v = sig_f*c (GPSIMD,
    parallel), c' = u+v in column halves (DVE); c state in bf16.
    h' = (sig_o*4)*tanh(c') via one STT, written fp8 in column halves so
    kk-major consumer matmuls start on the first half.
  - No transposes for h anywhere: matmul outputs land directly in the
    [hidden-part, (k-slice, batch)] layout the next matmul consumes.
  - Program order tuned for the greedy ready-first scheduler: next-step
    G1/G2 h-rounds and bias/const fills sit in the step tail and drain
    into PE idle windows; G2 h2-rounds are positioned after the G1 x-close
    so they cannot queue ahead of it.

Measured (CoreSim TRN2 cost model, per core): 321.3us (5.02us/step) vs
819.0us baseline (2.55x).  Backend-validated (8-core PJRT): rel err
4.17e-3 (tolerance 2e-2).  The loop is latency-bound on the per-step
dependency cycle argmax -> onehot -> L1 -> L2 -> fc2 -> fc3 -> argmax;
PE busy is only ~30%, all engines start ops as soon as data lands.
"""
import sys
import numpy as np

sys.path.insert(0, "/opt/trn_rl_repo")

import os
HIDDEN = 512
OUT = 33
T_STEPS = int(os.environ.get("KERNEL_STEPS", "64"))
B_FULL = 256
N_CORES = 8
B = B_FULL // N_CORES  # 32
SLOPE = 0.01
N_FILL_A = int(os.environ.get("N_FILL_A", "0"))
N_FILL_B = int(os.environ.get("N_FILL_B", "0"))
N_FILL_C = int(os.environ.get("N_FILL_C", "0"))
N_FILL_T = int(os.environ.get("N_FILL_T", "0"))
N_FILL_O = int(os.environ.get("N_FILL_O", "0"))
S_W = 8.0    # fp8 weight scale
S_H = 4.0    # fp8 hidden-state scale
SG = S_W * S_H  # 32: gate-psum scale

_PROGRAM_CACHE = {}
LAST_EXEC_NS = None

_BF16_NAMES = {"B2T", "CC2T", "F3rep", "I32", "c10", "c20"}
_FP8_NAMES = {"Whh1p", "Wih2p", "Whh2p", "fc2Wp", "W3p", "h1T0", "h2T0",
              "E1q", "oh0P"}

# chunk order within each gate tensor: go-tile chunks then if-tile chunks
_QORDER = ("g", "o", "i", "f")  # chunks 0..3=g, 4..7=o, 8..11=i, 12..15=f
_TBASE = {"i": 0, "f": 512, "g": 1024, "o": 1536}  # torch gate row blocks


def _bf16np():
    import ml_dtypes
    return ml_dtypes.bfloat16


def _fp8np():
    import ml_dtypes
    return ml_dtypes.float8_e4m3fn


def _chunk_rows(c):
    """Torch-row indices for chunk c (128 gate rows)."""
    q = _QORDER[c // 4]
    r = c % 4
    return np.arange(_TBASE[q] + 128 * r, _TBASE[q] + 128 * r + 128)


def _pack_gate_w(W):
    """[2048, 512] -> fp8 lhsT pack [128(k), 16(chunk), 2(kk), 2(pair), 128(M)]."""
    out = np.zeros((128, 16, 2, 2, 128), np.float32)
    for c in range(16):
        rows = _chunk_rows(c)
        for kk in range(2):
            for i in range(2):
                k0 = 128 * (2 * kk + i)
                # lhsT[p, m] = W[rows[m], k0+p]
                out[:, c, kk, i, :] = W[rows][:, k0:k0 + 128].T
    return (out * S_W)


def _prep(inputs):
    f32 = np.float32
    emb = np.asarray(inputs["emb"], f32)
    Wih = np.asarray(inputs["Wih"], f32)
    Whh = np.asarray(inputs["Whh"], f32)
    bih = np.asarray(inputs["bih"], f32)
    bhh = np.asarray(inputs["bhh"], f32)
    fcW = np.asarray(inputs["fcW"], f32)
    fcb = np.asarray(inputs["fcb"], f32)
    fc2W = np.asarray(inputs["fc2W"], f32)
    fc2b = np.asarray(inputs["fc2b"], f32)
    fc3W = np.asarray(inputs["fc3W"], f32)
    fc3b = np.asarray(inputs["fc3b"], f32)
    h0 = np.asarray(inputs["h0"], f32)
    c0 = np.asarray(inputs["c0"], f32)
    conditionals = np.asarray(inputs["conditionals"], f32)

    g = {}
    g["Whh1p"] = _pack_gate_w(Whh[0])
    g["Wih2p"] = _pack_gate_w(Wih[1])
    g["Whh2p"] = _pack_gate_w(Whh[1])

    # E1ext: x-path lookup table for layer 1 (bf16, scaled by SG)
    # col c*128+m -> torch gate row _chunk_rows(c)[m]
    colrows = np.concatenate([_chunk_rows(c) for c in range(16)])  # [2048]
    Wih1r = Wih[0][colrows]                       # [2048, 512]
    e1 = np.zeros((34, 2048), f32)
    e1[:32] = emb @ Wih1r[:, :511].T
    e1[32] = Wih1r[:, 511]
    e1[33] = (bih[0] + bhh[0])[colrows]
    # fp8 DoubleRow pack: slot 0 = class rows, slot 1 = dur/bias rows at
    # partitions 0/1 (matching ohP's constant slot-1 layout)
    e1q = np.zeros((32, 2, 2048), f32)
    e1q[:, 0, :] = e1[:32] * SG
    e1q[0, 1, :] = e1[32] * SG
    e1q[1, 1, :] = e1[33] * SG
    g["E1q"] = e1q

    b2 = (bih[1] + bhh[1])[colrows]               # [2048] chunk-major
    g["B2T"] = np.tile((b2 * SG)[None, :], (B, 1))  # [32, 2048]

    # fc2W pack: [128(k), 4(j), 2(kk), 2(pair), 128(m)]
    w2 = np.zeros((128, 4, 2, 2, 128), f32)
    for j in range(4):
        for kk in range(2):
            for i in range(2):
                k0 = 128 * (2 * kk + i)
                w2[:, j, kk, i, :] = fc2W[128 * j:128 * (j + 1), k0:k0 + 128].T
    g["fc2Wp"] = w2 * 2.0

    # fc3W pack: [128(k), 2(kk), 2(pair), 33]
    w3 = np.zeros((128, 2, 2, OUT), f32)
    for kk in range(2):
        for i in range(2):
            k0 = 128 * (2 * kk + i)
            w3[:, kk, i, :] = fc3W[:, k0:k0 + 128].T
    g["W3p"] = w3 * 4.0

    g["F3rep"] = np.tile(fc3b[None, :] * 32.0, (B, 1))
    g["I32"] = np.eye(32, dtype=f32)

    oh0 = np.zeros((32, 2, B), f32)
    oh0[0, 0, :] = 1.0  # SOS onehot
    oh0[0, 1, :] = 0.0  # dur at t=0
    oh0[1, 1, :] = 1.0  # bias row
    g["oh0P"] = oh0

    cond = conditionals @ fcW.T + fcb
    cond = np.where(cond >= 0, cond, SLOPE * cond).astype(f32)
    CC2 = (cond @ fc2W.T + fc2b).astype(f32)      # [256, 512]

    per_core = []
    for ci in range(N_CORES):
        sl = slice(ci * B, (ci + 1) * B)
        pc = {}
        for l, name in ((0, "h1T0"), (1, "h2T0")):
            hc = h0[l, sl]                        # [32, 512]
            # hT[p, 32k+b] = S_H * h[b, 128k+p]
            pc[name] = np.ascontiguousarray(
                hc.reshape(B, 4, 128).transpose(2, 1, 0).reshape(128, 128)) * S_H
        for l, name in ((0, "c10"), (1, "c20")):
            cc = c0[l, sl]
            pc[name] = np.ascontiguousarray(
                cc.reshape(B, 4, 128).transpose(2, 1, 0).reshape(128, 128))
        # CC2T[b, j*128+m] = SG * CC2[b, 128j+m]
        pc["CC2T"] = np.ascontiguousarray(CC2[sl] * 8.0)
        per_core.append(pc)
    return g, per_core


# ---------------------------------------------------------------------------
# Bass program
# ---------------------------------------------------------------------------
def _region(tile_go, tile_if, c):
    """PSUM region AP for chunk c: tile_go holds g|o, tile_if holds i|f."""
    r = c % 4
    if c < 4:
        return tile_go[:, 32 * r:32 * r + 32]
    if c < 8:
        return tile_go[:, 128 + 32 * r:128 + 32 * r + 32]
    if c < 12:
        return tile_if[:, 32 * r:32 * r + 32]
    return tile_if[:, 128 + 32 * r:128 + 32 * r + 32]


def _build_program():
    import concourse.bass as bass
    import concourse.tile as tile
    from concourse import mybir, bacc

    F32 = mybir.dt.float32
    BF16 = mybir.dt.bfloat16
    FP8 = mybir.dt.float8e4
    AF = mybir.ActivationFunctionType
    ALU = mybir.AluOpType
    DR = mybir.MatmulPerfMode.DoubleRow

    nc = bacc.Bacc("TRN2", target_bir_lowering=False, debug=False)

    def din(name, shape):
        dt = FP8 if name in _FP8_NAMES else (BF16 if name in _BF16_NAMES else F32)
        return nc.dram_tensor(name, list(shape), dt, kind="ExternalInput").ap()

    d = {
        "Whh1p": din("Whh1p", (128, 16, 2, 2, 128)),
        "Wih2p": din("Wih2p", (128, 16, 2, 2, 128)),
        "Whh2p": din("Whh2p", (128, 16, 2, 2, 128)),
        "fc2Wp": din("fc2Wp", (128, 4, 2, 2, 128)),
        "W3p": din("W3p", (128, 2, 2, OUT)),
        "E1q": din("E1q", (32, 2, 2048)),
        "B2T": din("B2T", (B, 2048)),
        "CC2T": din("CC2T", (B, 512)),
        "F3rep": din("F3rep", (B, OUT)),
        "I32": din("I32", (32, 32)),
        "oh0P": din("oh0P", (32, 2, B)),
        "h1T0": din("h1T0", (128, 128)),
        "h2T0": din("h2T0", (128, 128)),
        "c10": din("c10", (128, 128)),
        "c20": din("c20", (128, 128)),
    }
    out_d = nc.dram_tensor("out", [B, 64, OUT], F32, kind="ExternalOutput").ap()

    with tile.TileContext(nc) as tc:
        import contextlib
        ctx = contextlib.ExitStack()
        with ctx:
            consts = ctx.enter_context(tc.tile_pool(name="consts", bufs=1))
            state = ctx.enter_context(tc.tile_pool(name="state", bufs=1))
            work = ctx.enter_context(tc.tile_pool(name="work", bufs=2))
            hpool = ctx.enter_context(tc.tile_pool(name="hpool", bufs=2))
            ps_g1 = ctx.enter_context(tc.tile_pool(name="ps_g1", bufs=1, space="PSUM"))
            ps_g2 = ctx.enter_context(tc.tile_pool(name="ps_g2", bufs=1, space="PSUM"))
            ps_f = ctx.enter_context(tc.tile_pool(name="ps_f", bufs=1, space="PSUM"))
            ps_p3 = ctx.enter_context(tc.tile_pool(name="ps_p3", bufs=1, space="PSUM"))
            ps_fz = ctx.enter_context(tc.tile_pool(name="ps_fz", bufs=1, space="PSUM"))
            ps_fill = ctx.enter_context(tc.tile_pool(name="ps_fill", bufs=1, space="PSUM"))

            # ---- constant tiles ----
            I32 = consts.tile([32, 32], BF16)
            Whh1p = consts.tile([128, 16, 2, 2, 128], FP8)
            Wih2p = consts.tile([128, 16, 2, 2, 128], FP8)
            Whh2p = consts.tile([128, 16, 2, 2, 128], FP8)
            fc2Wp = consts.tile([128, 4, 2, 2, 128], FP8)
            W3p = consts.tile([128, 2, 2, OUT], FP8)
            E1q = consts.tile([32, 2, 2048], FP8)
            B2T = consts.tile([B, 2048], BF16)
            CC2T = consts.tile([B, 512], BF16)
            F3rep = consts.tile([B, OUT], BF16)
            oh0P = consts.tile([32, 2, B], FP8)

            c1 = state.tile([128, 128], BF16, tag="c1")
            c2 = state.tile([128, 128], BF16, tag="c2")
            h1 = hpool.tile([128, 128], FP8, tag="h1")
            h2 = hpool.tile([128, 128], FP8, tag="h2")
            ohP = state.tile([32, 2, B], FP8, tag="ohP")

            # DMAs: first-use order, spread across queues
            nc.sync.dma_start(I32[:], d["I32"])
            nc.sync.dma_start(h1[:], d["h1T0"])
            nc.sync.dma_start(c1[:], d["c10"])
            nc.sync.dma_start(oh0P[:], d["oh0P"])
            nc.sync.dma_start(E1q[:], d["E1q"])
            nc.sync.dma_start(Whh1p[:], d["Whh1p"])
            nc.gpsimd.dma_start(h2[:], d["h2T0"])
            nc.gpsimd.dma_start(c2[:], d["c20"])
            nc.gpsimd.dma_start(B2T[:], d["B2T"])
            nc.gpsimd.dma_start(Whh2p[:], d["Whh2p"])
            nc.scalar.dma_start(Wih2p[:], d["Wih2p"])
            nc.scalar.dma_start(CC2T[:], d["CC2T"])
            nc.scalar.dma_start(fc2Wp[:], d["fc2Wp"])
            nc.scalar.dma_start(W3p[:], d["W3p"])
            nc.scalar.dma_start(F3rep[:], d["F3rep"])

            nc.vector.memset(ohP[:, 1, :], 0.0)
            nc.vector.memset(ohP[0:2, 1, :], 1.0)

            predbuf = state.tile([B, 64, OUT], F32, tag="predbuf")
            if T_STEPS < 64:
                nc.vector.memset(predbuf[:], 0.0)

            def gate_rounds(Gg, Gi, Wp, hT, start):
                """32 DoubleRow h-rounds for one gate tensor.

                Each PSUM tile is bank-aligned (own zero region), so when
                `start` the first matmul into EACH tile opens that tile's
                accumulation group.
                """
                for c in range(16):
                    reg = _region(Gg, Gi, c)
                    for kk in range(2):
                        nc.tensor.matmul(
                            reg, Wp[:, c, kk],
                            hT[:, 64 * kk:64 * kk + 64].rearrange(
                                "p (two b) -> p two b", two=2),
                            start=(start and kk == 0 and c in (0, 8)),
                            stop=False, perf_mode=DR,
                            skip_group_check=True)

            def bias_rounds(Gg, Gi):
                """16 bf16 identity rounds adding B2; opens each tile's group."""
                for c in range(16):
                    reg = _region(Gg, Gi, c)
                    nc.tensor.matmul(reg, B2T[:, 128 * c:128 * (c + 1)], I32[:],
                                     start=(c in (0, 8)), stop=False,
                                     skip_group_check=True)

            def x_rounds(Gg, Gi, ohs):
                """16 fp8 DoubleRow E1 rounds; closes each tile."""
                for c in range(16):
                    reg = _region(Gg, Gi, c)
                    nc.tensor.matmul(reg, E1q[:, :, 128 * c:128 * (c + 1)], ohs,
                                     start=False, stop=(c in (7, 15)),
                                     perf_mode=DR, skip_group_check=True)

            def g2x_rounds(Gg, Gi, h1T):
                """32 DoubleRow Wih2 rounds, kk-major; closes each G2 tile."""
                for kk in range(2):
                    for c in range(16):
                        reg = _region(Gg, Gi, c)
                        nc.tensor.matmul(
                            reg, Wih2p[:, c, kk],
                            h1T[:, 64 * kk:64 * kk + 64].rearrange(
                                "p (two b) -> p two b", two=2),
                            start=False, stop=(c in (7, 15) and kk == 1),
                            perf_mode=DR, skip_group_check=True)

            def nonlin(layer, Gg, Gi, c_own):
                gt = work.tile([128, 128], BF16, tag=f"gt{layer}")
                nc.scalar.activation(gt[:], Gg[:, 0:128], AF.Tanh, scale=1.0 / SG)
                sif = work.tile([128, 256], BF16, tag=f"sif{layer}")
                nc.scalar.activation(sif[:], Gi[:], AF.Sigmoid, scale=1.0 / SG)
                u = work.tile([128, 128], BF16, tag=f"u{layer}")
                nc.vector.tensor_tensor(u[:], sif[:, 0:128], gt[:], ALU.mult)
                v = work.tile([128, 128], BF16, tag=f"v{layer}")
                nc.gpsimd.tensor_tensor(v[:], sif[:, 128:256], c_own[:], ALU.mult)
                nc.vector.tensor_tensor(c_own[:, 0:64], u[:, 0:64],
                                        v[:, 0:64], ALU.add)
                nc.vector.tensor_tensor(c_own[:, 64:128], u[:, 64:128],
                                        v[:, 64:128], ALU.add)
                so = work.tile([128, 128], BF16, tag=f"so{layer}")
                nc.scalar.activation(so[:], Gg[:, 128:256], AF.Sigmoid,
                                     scale=1.0 / SG)
                tct = work.tile([128, 128], BF16, tag=f"tc{layer}")
                nc.scalar.activation(tct[:], c_own[:], AF.Tanh)
                if N_FILL_T:
                    fillers_gen(N_FILL_T, tct[:, 0:32], gt[:, 0:64])
                hn = hpool.tile([128, 128], FP8, tag=f"h{layer}")
                # halves: kk-pair 0 (cols 0:64) lands first so kk-major
                # consumer matmuls start before the second half is done
                nc.vector.scalar_tensor_tensor(hn[:, 0:64], so[:, 0:64], S_H,
                                               tct[:, 0:64],
                                               op0=ALU.mult, op1=ALU.mult)
                nc.vector.scalar_tensor_tensor(hn[:, 64:128], so[:, 64:128],
                                               S_H, tct[:, 64:128],
                                               op0=ALU.mult, op1=ALU.mult)
                return hn

            def fc2_cc2(F, Fz):
                for T_ in (F, Fz):
                    for j in range(4):
                        nc.tensor.matmul(T_[:, 32 * j:32 * j + 32],
                                         CC2T[:, 128 * j:128 * (j + 1)], I32[:],
                                         start=(j == 0), stop=False,
                                         skip_group_check=True)

            def fc2_rounds(F, Fz, h2T):
                # twin PSUM targets: the relu branch (DVE) reads F while the
                # linear branch (ACT copy) reads Fz in parallel
                for T_ in (F, Fz):
                    for kk in range(2):
                        for j in range(4):
                            nc.tensor.matmul(
                                T_[:, 32 * j:32 * j + 32], fc2Wp[:, j, kk],
                                h2T[:, 64 * kk:64 * kk + 64].rearrange(
                                    "p (two b) -> p two b", two=2),
                                start=False, stop=(j == 3 and kk == 1),
                                perf_mode=DR, skip_group_check=True)

            # ---- t=0 preamble fills ----
            G1g = ps_g1.tile([128, 256], F32, tag="G1g")
            G1i = ps_g1.tile([128, 256], F32, tag="G1i")
            gate_rounds(G1g, G1i, Whh1p, h1, start=True)
            G2g = ps_g2.tile([128, 256], F32, tag="G2g")
            G2i = ps_g2.tile([128, 256], F32, tag="G2i")
            bias_rounds(G2g, G2i)
            gate_rounds(G2g, G2i, Whh2p, h2, start=False)
            F = ps_f.tile([128, 128], F32, tag="F")
            Fz = ps_fz.tile([128, 128], F32, tag="Fz")
            # PE p-state warmup
            for i in range(4):
                nc.tensor.matmul(F[0:32, 0:32], I32[:], I32[:], start=True,
                                 stop=True, skip_group_check=True)

            # p-state fillers: junk matmuls that keep the PE busy through the
            # chain's idle windows so chain matmuls are costed at the full
            # clock (the cost model's ramp tracks the last idle->busy edge).
            # Serialized via W-W deps on one PSUM tile, so at most one filler
            # ever sits ahead of real work (~27-53ns preemption delay).
            fill_t = ps_fill.tile([32, 64], F32, tag="fill")

            def fillers(n, dep_fp8_lhsT):
                for _ in range(n):
                    nc.tensor.matmul(fill_t[:], dep_fp8_lhsT,
                                     Whh1p[:, 0, 0, :, 0:64], start=True,
                                     stop=True, perf_mode=DR,
                                     skip_group_check=True)

            def fillers_gen(n, lhsT, rhs):
                for _ in range(n):
                    nc.tensor.matmul(fill_t[:, 0:64], lhsT, rhs, start=True,
                                     stop=True, skip_group_check=True)

            def fillers34(n, dep_lhsT_34):
                for _ in range(n):
                    nc.tensor.matmul(fill_t[:], dep_lhsT_34,
                                     E1q[:, :, 0:64], start=True,
                                     stop=True, perf_mode=DR,
                                     skip_group_check=True)
            fc2_cc2(F, Fz)
            p3 = ps_p3.tile([B, OUT], F32, tag="p3")
            nc.tensor.matmul(p3[:], I32[:], F3rep[:], start=True, stop=False,
                             skip_group_check=True)

            for t in range(T_STEPS):
                tb = t % 64
                ohs = oh0P if t == 0 else ohP
                # close G1
                x_rounds(G1g, G1i, ohs[:])
                fillers34(N_FILL_A, ohs[:])
                # G2 h2-rounds for THIS step: positioned after the G1x close
                # so they cannot queue ahead of it (in-order PE queue), but
                # they drain during the L1 chain window
                if t > 0:
                    gate_rounds(G2g, G2i, Whh2p, h2, start=False)
                # L1 chain
                h1 = nonlin(1, G1g, G1i, c1)
                # close G2
                g2x_rounds(G2g, G2i, h1)
                fillers(N_FILL_B, h1[:, 0:64].rearrange(
                    "p (two b) -> p two b", two=2))
                # L2 chain
                h2 = nonlin(2, G2g, G2i, c2)
                # fc2 close
                fc2_rounds(F, Fz, h2)
                # tail: leaky split into relu and linear branches
                rb = work.tile([128, 128], FP8, tag="rb")
                nc.vector.tensor_scalar(rb[:], F[:], 0.0, float(1.0 - SLOPE),
                                        op0=ALU.max, op1=ALU.mult)
                zb = work.tile([128, 128], FP8, tag="zb")
                nc.scalar.mul(zb[:], Fz[:], SLOPE)
                fillers(N_FILL_C, rb[:, 0:64].rearrange(
                    "p (two b) -> p two b", two=2))
                p3_cur, F_cur = p3, F
                for kk in range(2):
                    nc.tensor.matmul(
                        p3_cur[:],
                        rb[:, 64 * kk:64 * kk + 64].rearrange(
                            "p (two b) -> p two b", two=2),
                        W3p[:, kk], start=False, stop=False,
                        perf_mode=DR, skip_group_check=True)
                for kk in range(2):
                    nc.tensor.matmul(
                        p3_cur[:],
                        zb[:, 64 * kk:64 * kk + 64].rearrange(
                            "p (two b) -> p two b", two=2),
                        W3p[:, kk], start=False, stop=(kk == 1),
                        perf_mode=DR, skip_group_check=True)
                if t == T_STEPS - 1:
                    # ACT switches to the exp/ln table after the loop's last
                    # Tanh; hide the 1.3us load under the remaining PE work
                    dummy = work.tile([B, 1], F32, tag="dummy")
                    nc.scalar.activation(dummy[:], c2[0:32, 0:1], AF.Exp)
                # argmax feedback
                if t < T_STEPS - 1:
                    mx = work.tile([B, 8], F32, tag="mx")
                    nc.vector.max(mx[:], p3_cur[:, 0:32])
                    oh = work.tile([B, 32], FP8, tag="oh")
                    nc.vector.tensor_scalar(oh[:], p3_cur[:, 0:32],
                                            mx[:, 0:1], None, op0=ALU.is_equal)
                    if N_FILL_O:
                        fillers_gen(N_FILL_O, oh[:],
                                    oh0P[:].rearrange("p a b -> p (a b)"))
                    nc.vector.transpose(ohP[:, 0, :], oh[:])
                # pred copy (unscale by 1/32) on DVE after the argmax ops
                # (gpsimd cannot read PSUM; ACT would block next gate acts)
                nc.vector.tensor_scalar(predbuf[:, tb, :], p3_cur[:],
                                        1.0 / 32.0, None, op0=ALU.mult)
                # ---- fills for t+1 ----
                if t + 1 < T_STEPS:
                    G1g = ps_g1.tile([128, 256], F32, tag="G1g")
                    G1i = ps_g1.tile([128, 256], F32, tag="G1i")
                    gate_rounds(G1g, G1i, Whh1p, h1, start=True)
                    G2g = ps_g2.tile([128, 256], F32, tag="G2g")
                    G2i = ps_g2.tile([128, 256], F32, tag="G2i")
                    bias_rounds(G2g, G2i)
                    F = ps_f.tile([128, 128], F32, tag="F")
                    Fz = ps_fz.tile([128, 128], F32, tag="Fz")
                    fc2_cc2(F, Fz)
                    p3 = ps_p3.tile([B, OUT], F32, tag="p3")
                    nc.tensor.matmul(p3[:], I32[:], F3rep[:], start=True,
                                     stop=False, skip_group_check=True)

            # gate tile: forces postprocess exps after the loop
            gate0 = work.tile([B, 1], F32, tag="gate0")
            nc.vector.tensor_scalar(gate0[:], predbuf[:, T_STEPS - 1, 0:1],
                                    0.0, None, op0=ALU.mult)

            # ---- postprocess ----
            e = state.tile([B, 64, OUT], F32, tag="e")
            s = work.tile([B, 64], F32, tag="s")
            for t0 in range(0, 64, 32):
                nc.scalar.activation(e[:, t0:t0 + 32, :],
                                     predbuf[:, t0:t0 + 32, :], AF.Exp,
                                     bias=gate0[:, 0:1])
                nc.vector.tensor_reduce(s[:, t0:t0 + 32],
                                        e[:, t0:t0 + 32, 0:32],
                                        mybir.AxisListType.X, ALU.add)
            lns = work.tile([B, 64], F32, tag="lns")
            nc.scalar.activation(lns[:, 0:32], s[:, 0:32], AF.Ln)
            nc.scalar.activation(lns[:, 32:64], s[:, 32:64], AF.Ln)
            outf = state.tile([B, 64, OUT], F32, tag="outf")
            sd = work.tile([B, 1], F32, tag="sd")
            nc.vector.tensor_reduce(sd[:], e[:, :, 32:33], mybir.AxisListType.XY,
                                    ALU.add)
            rsd = work.tile([B, 1], F32, tag="rsd")
            nc.vector.reciprocal(rsd[:], sd[:])
            nc.gpsimd.tensor_scalar(outf[:, :, 32:33], e[:, :, 32:33],
                                    rsd[:, 0:1], None, op0=ALU.mult)
            for i, t0 in enumerate(range(0, 64, 16)):
                eng = nc.vector if i % 2 == 0 else nc.gpsimd
                eng.tensor_tensor(
                    outf[:, t0:t0 + 16, 0:32], predbuf[:, t0:t0 + 16, 0:32],
                    lns[:, t0:t0 + 16].broadcast_to((B, 16, 32)),
                    ALU.subtract)
                (nc.sync if i % 2 == 0 else nc.scalar).dma_start(
                    out_d[:, t0:t0 + 16, :], outf[:, t0:t0 + 16, :])

    nc.compile()
    return nc, out_d.tensor.name


def kernel(**inputs):
    from concourse import bass_utils

    g, per_core = _prep(inputs)
    if "prog" not in _PROGRAM_CACHE:
        _PROGRAM_CACHE["prog"] = _build_program()
    nc, out_name = _PROGRAM_CACHE["prog"]

    bf16, fp8 = _bf16np(), _fp8np()

    def conv(k, v):
        a = np.asarray(v, np.float32)
        if k in _FP8_NAMES:
            return np.ascontiguousarray(a.astype(fp8))
        if k in _BF16_NAMES:
            return np.ascontiguousarray(a.astype(bf16))
        return np.ascontiguousarray(a)

    in_maps = []
    for ci in range(N_CORES):
        m = dict(g)
        m.update(per_core[ci])
        in_maps.append({k: conv(k, v) for k, v in m.items()})
    ncores = int(os.environ.get("KERNEL_CORES", str(N_CORES)))
    kwargs = {}
    if os.environ.get("KERNEL_TRACE"):
        kwargs = dict(trace=True, tmpdir=os.environ.get("KERNEL_TRACE_DIR") or None)
    res = bass_utils.run_bass_kernel_spmd(nc, in_maps[:ncores],
                                          core_ids=list(range(ncores)), **kwargs)
    global LAST_EXEC_NS
    LAST_EXEC_NS = res.exec_time_ns
    out = np.concatenate([r[out_name] for r in res.results], axis=0)
    return out.astype(np.float32)


# revision 8
# speedup vs baseline: 1.0019x; 1.0019x over previous
"""Trainium2 Bass kernel for nn_Decoder_76974403879078 — v2 (weight-stationary).

2-layer LSTM decoder, B=256, H=512, T=64 steps, argmax feedback.
Sharding: data-parallel over batch, 8 cores x 32; the sequential time loop
runs locally per core (no collectives).

Design vs the 819us baseline (3439us fp32 original):
  - Weight-stationary matmuls: weights are the PE stationary operand
    (lhsT [K=128, M=128]), h the moving operand [K=128, N=32].  The cost
    model charges out-free-size x cycles/row, so streaming 32 batch cols
    instead of 512 gate cols cuts PE stream time ~4x.
  - fp8 e4m3 DoubleRow everywhere big: gate weights, fc2W, fc3W, the E1
    x-path table, and the h states.  One DoubleRow instruction contracts
    two K=128 tiles at 0.5 cycles/row (8x less PE stream than the
    baseline's bf16 output-stationary scheme).  Host study: full-fp8
    trajectory rel err ~4e-3 vs the 2e-2 tolerance; argmax flips are
    benign near-ties (bf16 itself flips 220/16384 with rel 3.9e-4).
    Weights x8, h x4 dodge fp8 subnormals; the 1/32 unscale folds into
    the ACT gate sigmoid/tanh `scale`.
  - Gate PSUM layout per layer: tiles (g|o) and (i|f), each its own
    bank/zero-region, closed per tile so tanh(g) fires after the go-tile
    x-rounds and sigmoid(i,f) right after the if-tile's.
  - L1 x-path: E1ext table matmul with the onehot packed as K=32
    DoubleRow pairs; dur/bias rows ride pair-slot 1 at partitions 0/1
    (constant after t=0, memset once).  argmax feedback: DVE max ->
    is_equal -> 32x32 StreamTranspose writes pair-slot 0 in place.
  - Bias/const injects (B2, CC2, F3) are bf16 identity matmuls
    (lhsT=rows, rhs=I32) that also open each bank's accumulation group -
    no warm-PSUM hacks.
  - leaky(z) -> two fp8 branches on twin PSUM copies of fc2 (tile dep
    tracking serializes same-tile readers): relu*0.99 on DVE (max+mult
    tensor_scalar) || 0.01*z on ACT (copy w/ scale); fc3 = 4 DoubleRow
    matmuls sharing one W3.
  - c-update: u = sig_i*tanh_g (DVE bf16 2x), v = sig_f*c (GPSIMD,
    parallel), c' = u+v in column halves (DVE); c state in bf16.
    h' = (sig_o*4)*tanh(c') via one STT, written fp8 in column halves so
    kk-major consumer matmuls start on the first half.
  - No transposes for h anywhere: matmul outputs land directly in the
    [hidden-part, (k-slice, batch)] layout the next matmul consumes.
  - Program order tuned for the greedy ready-first scheduler: next-step
    G1/G2 h-rounds and bias/const fills sit in the step tail and drain
    into PE idle windows; G2 h2-rounds are positioned after the G1 x-close
    so they cannot queue ahead of it.

Measured (CoreSim TRN2 cost model, per core): 321.3us (5.02us/step) vs
819.0us baseline (2.55x).  Backend-validated (8-core PJRT): rel err
4.17e-3 (tolerance 2e-2).  The loop is latency-bound on the per-step
dependency cycle argmax -> onehot -> L1 -> L2 -> fc2 -> fc3 -> argmax;
PE busy is only ~30%, all engines start ops as soon as data lands.
"""
import sys
import numpy as np

sys.path.insert(0, "/opt/trn_rl_repo")

import os
HIDDEN = 512
OUT = 33
T_STEPS = int(os.environ.get("KERNEL_STEPS", "64"))
B_FULL = 256
N_CORES = 8
B = B_FULL // N_CORES  # 32
SLOPE = 0.01
N_FILL_A = int(os.environ.get("N_FILL_A", "0"))
N_FILL_B = int(os.environ.get("N_FILL_B", "0"))
N_FILL_C = int(os.environ.get("N_FILL_C", "0"))
N_FILL_T = int(os.environ.get("N_FILL_T", "0"))
N_FILL_O = int(os.environ.get("N_FILL_O", "0"))
S_W = 8.0    # fp8 weight scale
S_H = 4.0    # fp8 hidden-state scale
SG = S_W * S_H  # 32: gate-psum scale

_PROGRAM_CACHE = {}
LAST_EXEC_NS = None

_BF16_NAMES = {"B2T", "CC2T", "F3rep", "I32", "c10", "c20"}
_FP8_NAMES = {"Whh1p", "Wih2p", "Whh2p", "fc2Wp", "W3p", "h1T0", "h2T0",
              "E1q", "oh0P"}

# chunk order within each gate tensor: go-tile chunks then if-tile chunks
_QORDER = ("g", "o", "i", "f")  # chunks 0..3=g, 4..7=o, 8..11=i, 12..15=f
_TBASE = {"i": 0, "f": 512, "g": 1024, "o": 1536}  # torch gate row blocks


def _bf16np():
    import ml_dtypes
    return ml_dtypes.bfloat16


def _fp8np():
    import ml_dtypes
    return ml_dtypes.float8_e4m3fn


def _chunk_rows(c):
    """Torch-row indices for chunk c (128 gate rows)."""
    q = _QORDER[c // 4]
    r = c % 4
    return np.arange(_TBASE[q] + 128 * r, _TBASE[q] + 128 * r + 128)


def _pack_gate_w(W):
    """[2048, 512] -> fp8 lhsT pack [128(k), 16(chunk), 2(kk), 2(pair), 128(M)]."""
    out = np.zeros((128, 16, 2, 2, 128), np.float32)
    for c in range(16):
        rows = _chunk_rows(c)
        for kk in range(2):
            for i in range(2):
                k0 = 128 * (2 * kk + i)
                # lhsT[p, m] = W[rows[m], k0+p]
                out[:, c, kk, i, :] = W[rows][:, k0:k0 + 128].T
    return (out * S_W)


def _prep(inputs):
    f32 = np.float32
    emb = np.asarray(inputs["emb"], f32)
    Wih = np.asarray(inputs["Wih"], f32)
    Whh = np.asarray(inputs["Whh"], f32)
    bih = np.asarray(inputs["bih"], f32)
    bhh = np.asarray(inputs["bhh"], f32)
    fcW = np.asarray(inputs["fcW"], f32)
    fcb = np.asarray(inputs["fcb"], f32)
    fc2W = np.asarray(inputs["fc2W"], f32)
    fc2b = np.asarray(inputs["fc2b"], f32)
    fc3W = np.asarray(inputs["fc3W"], f32)
    fc3b = np.asarray(inputs["fc3b"], f32)
    h0 = np.asarray(inputs["h0"], f32)
    c0 = np.asarray(inputs["c0"], f32)
    conditionals = np.asarray(inputs["conditionals"], f32)

    g = {}
    g["Whh1p"] = _pack_gate_w(Whh[0])
    g["Wih2p"] = _pack_gate_w(Wih[1])
    g["Whh2p"] = _pack_gate_w(Whh[1])

    # E1ext: x-path lookup table for layer 1 (bf16, scaled by SG)
    # col c*128+m -> torch gate row _chunk_rows(c)[m]
    colrows = np.concatenate([_chunk_rows(c) for c in range(16)])  # [2048]
    Wih1r = Wih[0][colrows]                       # [2048, 512]
    e1 = np.zeros((34, 2048), f32)
    e1[:32] = emb @ Wih1r[:, :511].T
    e1[32] = Wih1r[:, 511]
    e1[33] = (bih[0] + bhh[0])[colrows]
    # fp8 DoubleRow pack: slot 0 = class rows, slot 1 = dur/bias rows at
    # partitions 0/1 (matching ohP's constant slot-1 layout)
    e1q = np.zeros((32, 2, 2048), f32)
    e1q[:, 0, :] = e1[:32] * SG
    e1q[0, 1, :] = e1[32] * SG
    e1q[1, 1, :] = e1[33] * SG
    g["E1q"] = e1q

    b2 = (bih[1] + bhh[1])[colrows]               # [2048] chunk-major
    g["B2T"] = np.tile((b2 * SG)[None, :], (B, 1))  # [32, 2048]

    # fc2W pack: [128(k), 4(j), 2(kk), 2(pair), 128(m)]
    w2 = np.zeros((128, 4, 2, 2, 128), f32)
    for j in range(4):
        for kk in range(2):
            for i in range(2):
                k0 = 128 * (2 * kk + i)
                w2[:, j, kk, i, :] = fc2W[128 * j:128 * (j + 1), k0:k0 + 128].T
    g["fc2Wp"] = w2 * 2.0

    # fc3W pack: [128(k), 2(kk), 2(pair), 33]
    w3 = np.zeros((128, 2, 2, OUT), f32)
    for kk in range(2):
        for i in range(2):
            k0 = 128 * (2 * kk + i)
            w3[:, kk, i, :] = fc3W[:, k0:k0 + 128].T
    g["W3p"] = w3 * 4.0

    g["F3rep"] = np.tile(fc3b[None, :] * 32.0, (B, 1))
    g["I32"] = np.eye(32, dtype=f32)

    oh0 = np.zeros((32, 2, B), f32)
    oh0[0, 0, :] = 1.0  # SOS onehot
    oh0[0, 1, :] = 0.0  # dur at t=0
    oh0[1, 1, :] = 1.0  # bias row
    g["oh0P"] = oh0

    cond = conditionals @ fcW.T + fcb
    cond = np.where(cond >= 0, cond, SLOPE * cond).astype(f32)
    CC2 = (cond @ fc2W.T + fc2b).astype(f32)      # [256, 512]

    per_core = []
    for ci in range(N_CORES):
        sl = slice(ci * B, (ci + 1) * B)
        pc = {}
        for l, name in ((0, "h1T0"), (1, "h2T0")):
            hc = h0[l, sl]                        # [32, 512]
            # hT[p, 32k+b] = S_H * h[b, 128k+p]
            pc[name] = np.ascontiguousarray(
                hc.reshape(B, 4, 128).transpose(2, 1, 0).reshape(128, 128)) * S_H
        for l, name in ((0, "c10"), (1, "c20")):
            cc = c0[l, sl]
            pc[name] = np.ascontiguousarray(
                cc.reshape(B, 4, 128).transpose(2, 1, 0).reshape(128, 128))
        # CC2T[b, j*128+m] = SG * CC2[b, 128j+m]
        pc["CC2T"] = np.ascontiguousarray(CC2[sl] * 8.0)
        per_core.append(pc)
    return g, per_core


# ---------------------------------------------------------------------------
# Bass program
# ---------------------------------------------------------------------------
def _region(tile_go, tile_if, c):
    """PSUM region AP for chunk c: tile_go holds g|o, tile_if holds i|f."""
    r = c % 4
    if c < 4:
        return tile_go[:, 32 * r:32 * r + 32]
    if c < 8:
        return tile_go[:, 128 + 32 * r:128 + 32 * r + 32]
    if c < 12:
        return tile_if[:, 32 * r:32 * r + 32]
    return tile_if[:, 128 + 32 * r:128 + 32 * r + 32]


def _build_program():
    import concourse.bass as bass
    import concourse.tile as tile
    from concourse import mybir, bacc

    F32 = mybir.dt.float32
    BF16 = mybir.dt.bfloat16
    FP8 = mybir.dt.float8e4
    AF = mybir.ActivationFunctionType
    ALU = mybir.AluOpType
    DR = mybir.MatmulPerfMode.DoubleRow

    nc = bacc.Bacc("TRN2", target_bir_lowering=False, debug=False)

    def din(name, shape):
        dt = FP8 if name in _FP8_NAMES else (BF16 if name in _BF16_NAMES else F32)
        return nc.dram_tensor(name, list(shape), dt, kind="ExternalInput").ap()

    d = {
        "Whh1p": din("Whh1p", (128, 16, 2, 2, 128)),
        "Wih2p": din("Wih2p", (128, 16, 2, 2, 128)),
        "Whh2p": din("Whh2p", (128, 16, 2, 2, 128)),
        "fc2Wp": din("fc2Wp", (128, 4, 2, 2, 128)),
        "W3p": din("W3p", (128, 2, 2, OUT)),
        "E1q": din("E1q", (32, 2, 2048)),
        "B2T": din("B2T", (B, 2048)),
        "CC2T": din("CC2T", (B, 512)),
        "F3rep": din("F3rep", (B, OUT)),
        "I32": din("I32", (32, 32)),
        "oh0P": din("oh0P", (32, 2, B)),
        "h1T0": din("h1T0", (128, 128)),
        "h2T0": din("h2T0", (128, 128)),
        "c10": din("c10", (128, 128)),
        "c20": din("c20", (128, 128)),
    }
    out_d = nc.dram_tensor("out", [B, 64, OUT], F32, kind="ExternalOutput").ap()

    with tile.TileContext(nc) as tc:
        import contextlib
        ctx = contextlib.ExitStack()
        with ctx:
            consts = ctx.enter_context(tc.tile_pool(name="consts", bufs=1))
            state = ctx.enter_context(tc.tile_pool(name="state", bufs=1))
            work = ctx.enter_context(tc.tile_pool(name="work", bufs=2))
            hpool = ctx.enter_context(tc.tile_pool(name="hpool", bufs=2))
            ps_g1 = ctx.enter_context(tc.tile_pool(name="ps_g1", bufs=1, space="PSUM"))
            ps_g2 = ctx.enter_context(tc.tile_pool(name="ps_g2", bufs=1, space="PSUM"))
            ps_f = ctx.enter_context(tc.tile_pool(name="ps_f", bufs=1, space="PSUM"))
            ps_p3 = ctx.enter_context(tc.tile_pool(name="ps_p3", bufs=1, space="PSUM"))
            ps_fz = ctx.enter_context(tc.tile_pool(name="ps_fz", bufs=1, space="PSUM"))
            ps_fill = ctx.enter_context(tc.tile_pool(name="ps_fill", bufs=1, space="PSUM"))

            # ---- constant tiles ----
            I32 = consts.tile([32, 32], BF16)
            Whh1p = consts.tile([128, 16, 2, 2, 128], FP8)
            Wih2p = consts.tile([128, 16, 2, 2, 128], FP8)
            Whh2p = consts.tile([128, 16, 2, 2, 128], FP8)
            fc2Wp = consts.tile([128, 4, 2, 2, 128], FP8)
            W3p = consts.tile([128, 2, 2, OUT], FP8)
            E1q = consts.tile([32, 2, 2048], FP8)
            B2T = consts.tile([B, 2048], BF16)
            CC2T = consts.tile([B, 512], BF16)
            F3rep = consts.tile([B, OUT], BF16)
            oh0P = consts.tile([32, 2, B], FP8)

            c1 = state.tile([128, 128], BF16, tag="c1")
            c2 = state.tile([128, 128], BF16, tag="c2")
            h1 = hpool.tile([128, 128], FP8, tag="h1")
            h2 = hpool.tile([128, 128], FP8, tag="h2")
            ohP = state.tile([32, 2, B], FP8, tag="ohP")

            # DMAs: first-use order, spread across queues
            nc.sync.dma_start(I32[:], d["I32"])
            nc.sync.dma_start(h1[:], d["h1T0"])
            nc.sync.dma_start(c1[:], d["c10"])
            nc.sync.dma_start(oh0P[:], d["oh0P"])
            nc.sync.dma_start(E1q[:], d["E1q"])
            nc.sync.dma_start(Whh1p[:], d["Whh1p"])
            nc.gpsimd.dma_start(h2[:], d["h2T0"])
            nc.gpsimd.dma_start(c2[:], d["c20"])
            nc.gpsimd.dma_start(B2T[:], d["B2T"])
            nc.gpsimd.dma_start(Whh2p[:], d["Whh2p"])
            nc.scalar.dma_start(Wih2p[:], d["Wih2p"])
            nc.scalar.dma_start(CC2T[:], d["CC2T"])
            nc.scalar.dma_start(fc2Wp[:], d["fc2Wp"])
            nc.scalar.dma_start(W3p[:], d["W3p"])
            nc.scalar.dma_start(F3rep[:], d["F3rep"])

            nc.vector.memset(ohP[:, 1, :], 0.0)
            nc.vector.memset(ohP[0:2, 1, :], 1.0)

            predbuf = state.tile([B, 64, OUT], F32, tag="predbuf")
            if T_STEPS < 64:
                nc.vector.memset(predbuf[:], 0.0)

            def gate_rounds(Gg, Gi, Wp, hT, start):
                """32 DoubleRow h-rounds for one gate tensor.

                Each PSUM tile is bank-aligned (own zero region), so when
                `start` the first matmul into EACH tile opens that tile's
                accumulation group.
                """
                for c in range(16):
                    reg = _region(Gg, Gi, c)
                    for kk in range(2):
                        nc.tensor.matmul(
                            reg, Wp[:, c, kk],
                            hT[:, 64 * kk:64 * kk + 64].rearrange(
                                "p (two b) -> p two b", two=2),
                            start=(start and kk == 0 and c in (0, 8)),
                            stop=False, perf_mode=DR,
                            skip_group_check=True)

            def bias_rounds(Gg, Gi):
                """16 bf16 identity rounds adding B2; opens each tile's group."""
                for c in range(16):
                    reg = _region(Gg, Gi, c)
                    nc.tensor.matmul(reg, B2T[:, 128 * c:128 * (c + 1)], I32[:],
                                     start=(c in (0, 8)), stop=False,
                                     skip_group_check=True)

            def x_rounds(Gg, Gi, ohs):
                """16 fp8 DoubleRow E1 rounds; closes each tile."""
                for c in range(16):
                    reg = _region(Gg, Gi, c)
                    nc.tensor.matmul(reg, E1q[:, :, 128 * c:128 * (c + 1)], ohs,
                                     start=False, stop=(c in (7, 15)),
                                     perf_mode=DR, skip_group_check=True)

            def g2x_rounds(Gg, Gi, h1T):
                """32 DoubleRow Wih2 rounds, kk-major; closes each G2 tile."""
                for kk in range(2):
                    for c in range(16):
                        reg = _region(Gg, Gi, c)
                        nc.tensor.matmul(
                            reg, Wih2p[:, c, kk],
                            h1T[:, 64 * kk:64 * kk + 64].rearrange(
                                "p (two b) -> p two b", two=2),
                            start=False, stop=(c in (7, 15) and kk == 1),
                            perf_mode=DR, skip_group_check=True)

            def nonlin(layer, Gg, Gi, c_own):
                gt = work.tile([128, 128], BF16, tag=f"gt{layer}")
                nc.scalar.activation(gt[:], Gg[:, 0:128], AF.Tanh, scale=1.0 / SG)
                sif = work.tile([128, 256], BF16, tag=f"sif{layer}")
                nc.scalar.activation(sif[:], Gi[:], AF.Sigmoid, scale=1.0 / SG)
                u = work.tile([128, 128], BF16, tag=f"u{layer}")
                nc.vector.tensor_tensor(u[:], sif[:, 0:128], gt[:], ALU.mult)
                v = work.tile([128, 128], BF16, tag=f"v{layer}")
                nc.gpsimd.tensor_tensor(v[:], sif[:, 128:256], c_own[:], ALU.mult)
                nc.vector.tensor_tensor(c_own[:, 0:64], u[:, 0:64],
                                        v[:, 0:64], ALU.add)
                nc.vector.tensor_tensor(c_own[:, 64:128], u[:, 64:128],
                                        v[:, 64:128], ALU.add)
                so = work.tile([128, 128], BF16, tag=f"so{layer}")
                nc.scalar.activation(so[:], Gg[:, 128:256], AF.Sigmoid,
                                     scale=1.0 / SG)
                tct = work.tile([128, 128], BF16, tag=f"tc{layer}")
                nc.scalar.activation(tct[:], c_own[:], AF.Tanh)
                if N_FILL_T:
                    fillers_gen(N_FILL_T, tct[:, 0:32], gt[:, 0:64])
                hn = hpool.tile([128, 128], FP8, tag=f"h{layer}")
                # halves: kk-pair 0 (cols 0:64) lands first so kk-major
                # consumer matmuls start before the second half is done
                nc.vector.scalar_tensor_tensor(hn[:, 0:64], so[:, 0:64], S_H,
                                               tct[:, 0:64],
                                               op0=ALU.mult, op1=ALU.mult)
                nc.vector.scalar_tensor_tensor(hn[:, 64:128], so[:, 64:128],
                                               S_H, tct[:, 64:128],
                                               op0=ALU.mult, op1=ALU.mult)
                return hn

            def fc2_cc2(F, Fz):
                for T_ in (F, Fz):
                    for j in range(4):
                        nc.tensor.matmul(T_[:, 32 * j:32 * j + 32],
                                         CC2T[:, 128 * j:128 * (j + 1)], I32[:],
                                         start=(j == 0), stop=False,
                                         skip_group_check=True)

            def fc2_rounds(F, Fz, h2T):
                # twin PSUM targets: the relu branch (DVE) reads F while the
                # linear branch (ACT copy) reads Fz in parallel
                for T_ in (F, Fz):
                    for kk in range(2):
                        for j in range(4):
                            nc.tensor.matmul(
                                T_[:, 32 * j:32 * j + 32], fc2Wp[:, j, kk],
                                h2T[:, 64 * kk:64 * kk + 64].rearrange(
                                    "p (two b) -> p two b", two=2),
                                start=False, stop=(j == 3 and kk == 1),
                                perf_mode=DR, skip_group_check=True)

            # ---- t=0 preamble fills ----
            G1g = ps_g1.tile([128, 256], F32, tag="G1g")
            G1i = ps_g1.tile([128, 256], F32, tag="G1i")
            gate_rounds(G1g, G1i, Whh1p, h1, start=True)
            G2g = ps_g2.tile([128, 256], F32, tag="G2g")
            G2i = ps_g2.tile([128, 256], F32, tag="G2i")
            bias_rounds(G2g, G2i)
            gate_rounds(G2g, G2i, Whh2p, h2, start=False)
            F = ps_f.tile([128, 128], F32, tag="F")
            Fz = ps_fz.tile([128, 128], F32, tag="Fz")
            # PE p-state warmup
            for i in range(4):
                nc.tensor.matmul(F[0:32, 0:32], I32[:], I32[:], start=True,
                                 stop=True, skip_group_check=True)

            # p-state fillers: junk matmuls that keep the PE busy through the
            # chain's idle windows so chain matmuls are costed at the full
            # clock (the cost model's ramp tracks the last idle->busy edge).
            # Serialized via W-W deps on one PSUM tile, so at most one filler
            # ever sits ahead of real work (~27-53ns preemption delay).
            fill_t = ps_fill.tile([32, 64], F32, tag="fill")

            def fillers(n, dep_fp8_lhsT):
                for _ in range(n):
                    nc.tensor.matmul(fill_t[:], dep_fp8_lhsT,
                                     Whh1p[:, 0, 0, :, 0:64], start=True,
                                     stop=True, perf_mode=DR,
                                     skip_group_check=True)

            def fillers_gen(n, lhsT, rhs):
                for _ in range(n):
                    nc.tensor.matmul(fill_t[:, 0:64], lhsT, rhs, start=True,
                                     stop=True, skip_group_check=True)

            def fillers34(n, dep_lhsT_34):
                for _ in range(n):
                    nc.tensor.matmul(fill_t[:], dep_lhsT_34,
                                     E1q[:, :, 0:64], start=True,
                                     stop=True, perf_mode=DR,
                                     skip_group_check=True)
            fc2_cc2(F, Fz)
            p3 = ps_p3.tile([B, OUT], F32, tag="p3")
            nc.tensor.matmul(p3[:], I32[:], F3rep[:], start=True, stop=False,
                             skip_group_check=True)

            for t in range(T_STEPS):
                tb = t % 64
                ohs = oh0P if t == 0 else ohP
                # close G1
                x_rounds(G1g, G1i, ohs[:])
                fillers34(N_FILL_A, ohs[:])
                # G2 h2-rounds for THIS step: positioned after the G1x close
                # so they cannot queue ahead of it (in-order PE queue), but
                # they drain during the L1 chain window
                if t > 0:
                    gate_rounds(G2g, G2i, Whh2p, h2, start=False)
                # L1 chain
                h1 = nonlin(1, G1g, G1i, c1)
                # close G2
                g2x_rounds(G2g, G2i, h1)
                fillers(N_FILL_B, h1[:, 0:64].rearrange(
                    "p (two b) -> p two b", two=2))
                # L2 chain
                h2 = nonlin(2, G2g, G2i, c2)
                # fc2 close
                fc2_rounds(F, Fz, h2)
                # tail: leaky split into relu and linear branches
                rb = work.tile([128, 128], FP8, tag="rb")
                nc.vector.tensor_scalar(rb[:], F[:], 0.0, float(1.0 - SLOPE),
                                        op0=ALU.max, op1=ALU.mult)
                zb = work.tile([128, 128], FP8, tag="zb")
                nc.scalar.mul(zb[:], Fz[:], SLOPE)
                fillers(N_FILL_C, rb[:, 0:64].rearrange(
                    "p (two b) -> p two b", two=2))
                p3_cur, F_cur = p3, F
                for kk in range(2):
                    nc.tensor.matmul(
                        p3_cur[:],
                        rb[:, 64 * kk:64 * kk + 64].rearrange(
                            "p (two b) -> p two b", two=2),
                        W3p[:, kk], start=False, stop=False,
                        perf_mode=DR, skip_group_check=True)
                for kk in range(2):
                    nc.tensor.matmul(
                        p3_cur[:],
                        zb[:, 64 * kk:64 * kk + 64].rearrange(
                            "p (two b) -> p two b", two=2),
                        W3p[:, kk], start=False, stop=(kk == 1),
                        perf_mode=DR, skip_group_check=True)
                if t == T_STEPS - 1:
                    # ACT switches to the exp/ln table after the loop's last
                    # Tanh; hide the 1.3us load under the remaining PE work
                    dummy = work.tile([B, 1], F32, tag="dummy")
                    nc.scalar.activation(dummy[:], c2[0:32, 0:1], AF.Exp)
                # argmax feedback
                if t < T_STEPS - 1:
                    mx = work.tile([B, 8], F32, tag="mx")
                    nc.vector.max(mx[:], p3_cur[:, 0:32])
                    oh = work.tile([B, 32], FP8, tag="oh")
                    nc.vector.tensor_scalar(oh[:], p3_cur[:, 0:32],
                                            mx[:, 0:1], None, op0=ALU.is_equal)
                    if N_FILL_O:
                        fillers_gen(N_FILL_O, oh[:],
                                    oh0P[:].rearrange("p a b -> p (a b)"))
                    nc.vector.transpose(ohP[:, 0, :], oh[:])
                # pred copy (unscale by 1/32) on DVE after the argmax ops
                # (gpsimd cannot read PSUM; ACT would block next gate acts)
                nc.vector.tensor_scalar(predbuf[:, tb, :], p3_cur[:],
                                        1.0 / 32.0, None, op0=ALU.mult)
                # ---- fills for t+1 ----
                if t + 1 < T_STEPS:
                    G1g = ps_g1.tile([128, 256], F32, tag="G1g")
                    G1i = ps_g1.tile([128, 256], F32, tag="G1i")
                    gate_rounds(G1g, G1i, Whh1p, h1, start=True)
                    G2g = ps_g2.tile([128, 256], F32, tag="G2g")
                    G2i = ps_g2.tile([128, 256], F32, tag="G2i")
                    bias_rounds(G2g, G2i)
                    F = ps_f.tile([128, 128], F32, tag="F")
                    Fz = ps_fz.tile([128, 128], F32, tag="Fz")
                    fc2_cc2(F, Fz)
                    p3 = ps_p3.tile([B, OUT], F32, tag="p3")
                    nc.tensor.matmul(p3[:], I32[:], F3rep[:], start=True,
                                     stop=False, skip_group_check=True)

            # gate tile: forces postprocess exps after the loop.  Derived from
            # the final c2 state (lands right after the loop's last c-update),
            # ~1us earlier than predbuf[63] - the first exp chunk only needs
            # predbuf[0:32], which is long done.
            gate0 = work.tile([B, 1], F32, tag="gate0")
            nc.vector.tensor_scalar(gate0[:], zb[0:B, 0:1],
                                    0.0, None, op0=ALU.mult)

            # ---- postprocess ----
            e = state.tile([B, 64, OUT], F32, tag="e")
            s = work.tile([B, 64], F32, tag="s")
            for t0 in range(0, 64, 32):
                nc.scalar.activation(e[:, t0:t0 + 32, :],
                                     predbuf[:, t0:t0 + 32, :], AF.Exp,
                                     bias=gate0[:, 0:1])
                nc.vector.tensor_reduce(s[:, t0:t0 + 32],
                                        e[:, t0:t0 + 32, 0:32],
                                        mybir.AxisListType.X, ALU.add)
            lns = work.tile([B, 64], F32, tag="lns")
            nc.scalar.activation(lns[:, 0:32], s[:, 0:32], AF.Ln)
            nc.scalar.activation(lns[:, 32:64], s[:, 32:64], AF.Ln)
            outf = state.tile([B, 64, OUT], F32, tag="outf")
            sd = work.tile([B, 1], F32, tag="sd")
            nc.vector.tensor_reduce(sd[:], e[:, :, 32:33], mybir.AxisListType.XY,
                                    ALU.add)
            rsd = work.tile([B, 1], F32, tag="rsd")
            nc.vector.reciprocal(rsd[:], sd[:])
            nc.gpsimd.tensor_scalar(outf[:, :, 32:33], e[:, :, 32:33],
                                    rsd[:, 0:1], None, op0=ALU.mult)
            dqs = (nc.sync, nc.scalar, nc.gpsimd)
            for i, t0 in enumerate(range(0, 64, 8)):
                eng = nc.vector if i % 2 == 0 else nc.gpsimd
                eng.tensor_tensor(
                    outf[:, t0:t0 + 8, 0:32], predbuf[:, t0:t0 + 8, 0:32],
                    lns[:, t0:t0 + 8].broadcast_to((B, 8, 32)),
                    ALU.subtract)
                dqs[i % 3].dma_start(
                    out_d[:, t0:t0 + 8, :], outf[:, t0:t0 + 8, :])

    nc.compile()
    return nc, out_d.tensor.name


def kernel(**inputs):
    from concourse import bass_utils

    g, per_core = _prep(inputs)
    if "prog" not in _PROGRAM_CACHE:
        _PROGRAM_CACHE["prog"] = _build_program()
    nc, out_name = _PROGRAM_CACHE["prog"]

    bf16, fp8 = _bf16np(), _fp8np()

    def conv(k, v):
        a = np.asarray(v, np.float32)
        if k in _FP8_NAMES:
            return np.ascontiguousarray(a.astype(fp8))
        if k in _BF16_NAMES:
            return np.ascontiguousarray(a.astype(bf16))
        return np.ascontiguousarray(a)

    in_maps = []
    for ci in range(N_CORES):
        m = dict(g)
        m.update(per_core[ci])
        in_maps.append({k: conv(k, v) for k, v in m.items()})
    ncores = int(os.environ.get("KERNEL_CORES", str(N_CORES)))
    kwargs = {}
    if os.environ.get("KERNEL_TRACE"):
        kwargs = dict(trace=True, tmpdir=os.environ.get("KERNEL_TRACE_DIR") or None)
    res = bass_utils.run_bass_kernel_spmd(nc, in_maps[:ncores],
                                          core_ids=list(range(ncores)), **kwargs)
    global LAST_EXEC_NS
    LAST_EXEC_NS = res.exec_time_ns
    out = np.concatenate([r[out_name] for r in res.results], axis=0)
    return out.astype(np.float32)


# revision 10
# speedup vs baseline: 1.0050x; 1.0030x over previous
"""Trainium2 Bass kernel for nn_Decoder_76974403879078 — v2 (weight-stationary).

2-layer LSTM decoder, B=256, H=512, T=64 steps, argmax feedback.
Sharding: data-parallel over batch, 8 cores x 32; the sequential time loop
runs locally per core (no collectives).

Design vs the 819us baseline (3439us fp32 original):
  - Weight-stationary matmuls: weights are the PE stationary operand
    (lhsT [K=128, M=128]), h the moving operand [K=128, N=32].  The cost
    model charges out-free-size x cycles/row, so streaming 32 batch cols
    instead of 512 gate cols cuts PE stream time ~4x.
  - fp8 e4m3 DoubleRow everywhere big: gate weights, fc2W, fc3W, the E1
    x-path table, and the h states.  One DoubleRow instruction contracts
    two K=128 tiles at 0.5 cycles/row (8x less PE stream than the
    baseline's bf16 output-stationary scheme).  Host study: full-fp8
    trajectory rel err ~4e-3 vs the 2e-2 tolerance; argmax flips are
    benign near-ties (bf16 itself flips 220/16384 with rel 3.9e-4).
    Weights x8, h x4 dodge fp8 subnormals; the 1/32 unscale folds into
    the ACT gate sigmoid/tanh `scale`.
  - Gate PSUM layout per layer: tiles (g|o) and (i|f), each its own
    bank/zero-region, closed per tile so tanh(g) fires after the go-tile
    x-rounds and sigmoid(i,f) right after the if-tile's.
  - L1 x-path: E1ext table matmul with the onehot packed as K=32
    DoubleRow pairs; dur/bias rows ride pair-slot 1 at partitions 0/1
    (constant after t=0, memset once).  argmax feedback: DVE max ->
    is_equal -> 32x32 StreamTranspose writes pair-slot 0 in place.
  - Bias/const injects (B2, CC2, F3) are bf16 identity matmuls
    (lhsT=rows, rhs=I32) that also open each bank's accumulation group -
    no warm-PSUM hacks.
  - leaky(z) -> two fp8 branches on twin PSUM copies of fc2 (tile dep
    tracking serializes same-tile readers): relu*0.99 on DVE (max+mult
    tensor_scalar) || 0.01*z on ACT (copy w/ scale); fc3 = 4 DoubleRow
    matmuls sharing one W3.
  - c-update: u = sig_i*tanh_g (DVE bf16 2x), v = sig_f*c (GPSIMD,
    parallel), c' = u+v in column halves (DVE); c state in bf16.
    h' = (sig_o*4)*tanh(c') via one STT, written fp8 in column halves so
    kk-major consumer matmuls start on the first half.
  - No transposes for h anywhere: matmul outputs land directly in the
    [hidden-part, (k-slice, batch)] layout the next matmul consumes.
  - Program order tuned for the greedy ready-first scheduler: next-step
    G1/G2 h-rounds and bias/const fills sit in the step tail and drain
    into PE idle windows; G2 h2-rounds are positioned after the G1 x-close
    so they cannot queue ahead of it.
  - Postprocess tail: exps gated on the last step's zb (the ACT queue's
    final loop op) rather than predbuf[63]; output DMA'd in 8 chunks over
    three DGE queues so the last transfer is short.

Measured (CoreSim TRN2 cost model, per core): 319.7us (5.00us/step) vs
819.0us baseline (2.55x).  Backend-validated (8-core PJRT): rel err
4.17e-3 (tolerance 2e-2).  The loop is latency-bound on the per-step
dependency cycle argmax -> onehot -> L1 -> L2 -> fc2 -> fc3 -> argmax;
PE busy is only ~30%, all engines start ops as soon as data lands.
"""
import sys
import numpy as np

sys.path.insert(0, "/opt/trn_rl_repo")

import os
HIDDEN = 512
OUT = 33
T_STEPS = int(os.environ.get("KERNEL_STEPS", "64"))
B_FULL = 256
N_CORES = 8
B = B_FULL // N_CORES  # 32
SLOPE = 0.01
N_FILL_A = int(os.environ.get("N_FILL_A", "0"))
N_FILL_B = int(os.environ.get("N_FILL_B", "0"))
N_FILL_C = int(os.environ.get("N_FILL_C", "0"))
N_FILL_T = int(os.environ.get("N_FILL_T", "0"))
N_FILL_O = int(os.environ.get("N_FILL_O", "0"))
S_W = 8.0    # fp8 weight scale
S_H = 4.0    # fp8 hidden-state scale
SG = S_W * S_H  # 32: gate-psum scale

_PROGRAM_CACHE = {}
LAST_EXEC_NS = None

_BF16_NAMES = {"B2T", "CC2T", "F3rep", "I32", "c10", "c20"}
_FP8_NAMES = {"Whh1p", "Wih2p", "Whh2p", "fc2Wp", "W3p", "h1T0", "h2T0",
              "E1q", "oh0P"}

# chunk order within each gate tensor: go-tile chunks then if-tile chunks
_QORDER = ("g", "o", "i", "f")  # chunks 0..3=g, 4..7=o, 8..11=i, 12..15=f
_TBASE = {"i": 0, "f": 512, "g": 1024, "o": 1536}  # torch gate row blocks


def _bf16np():
    import ml_dtypes
    return ml_dtypes.bfloat16


def _fp8np():
    import ml_dtypes
    return ml_dtypes.float8_e4m3fn


def _chunk_rows(c):
    """Torch-row indices for chunk c (128 gate rows)."""
    q = _QORDER[c // 4]
    r = c % 4
    return np.arange(_TBASE[q] + 128 * r, _TBASE[q] + 128 * r + 128)


def _pack_gate_w(W):
    """[2048, 512] -> fp8 lhsT pack [128(k), 16(chunk), 2(kk), 2(pair), 128(M)]."""
    out = np.zeros((128, 16, 2, 2, 128), np.float32)
    for c in range(16):
        rows = _chunk_rows(c)
        for kk in range(2):
            for i in range(2):
                k0 = 128 * (2 * kk + i)
                # lhsT[p, m] = W[rows[m], k0+p]
                out[:, c, kk, i, :] = W[rows][:, k0:k0 + 128].T
    return (out * S_W)


def _prep(inputs):
    f32 = np.float32
    emb = np.asarray(inputs["emb"], f32)
    Wih = np.asarray(inputs["Wih"], f32)
    Whh = np.asarray(inputs["Whh"], f32)
    bih = np.asarray(inputs["bih"], f32)
    bhh = np.asarray(inputs["bhh"], f32)
    fcW = np.asarray(inputs["fcW"], f32)
    fcb = np.asarray(inputs["fcb"], f32)
    fc2W = np.asarray(inputs["fc2W"], f32)
    fc2b = np.asarray(inputs["fc2b"], f32)
    fc3W = np.asarray(inputs["fc3W"], f32)
    fc3b = np.asarray(inputs["fc3b"], f32)
    h0 = np.asarray(inputs["h0"], f32)
    c0 = np.asarray(inputs["c0"], f32)
    conditionals = np.asarray(inputs["conditionals"], f32)

    g = {}
    g["Whh1p"] = _pack_gate_w(Whh[0])
    g["Wih2p"] = _pack_gate_w(Wih[1])
    g["Whh2p"] = _pack_gate_w(Whh[1])

    # E1ext: x-path lookup table for layer 1 (bf16, scaled by SG)
    # col c*128+m -> torch gate row _chunk_rows(c)[m]
    colrows = np.concatenate([_chunk_rows(c) for c in range(16)])  # [2048]
    Wih1r = Wih[0][colrows]                       # [2048, 512]
    e1 = np.zeros((34, 2048), f32)
    e1[:32] = emb @ Wih1r[:, :511].T
    e1[32] = Wih1r[:, 511]
    e1[33] = (bih[0] + bhh[0])[colrows]
    # fp8 DoubleRow pack: slot 0 = class rows, slot 1 = dur/bias rows at
    # partitions 0/1 (matching ohP's constant slot-1 layout)
    e1q = np.zeros((32, 2, 2048), f32)
    e1q[:, 0, :] = e1[:32] * SG
    e1q[0, 1, :] = e1[32] * SG
    e1q[1, 1, :] = e1[33] * SG
    g["E1q"] = e1q

    b2 = (bih[1] + bhh[1])[colrows]               # [2048] chunk-major
    g["B2T"] = np.tile((b2 * SG)[None, :], (B, 1))  # [32, 2048]

    # fc2W pack: [128(k), 4(j), 2(kk), 2(pair), 128(m)]
    w2 = np.zeros((128, 4, 2, 2, 128), f32)
    for j in range(4):
        for kk in range(2):
            for i in range(2):
                k0 = 128 * (2 * kk + i)
                w2[:, j, kk, i, :] = fc2W[128 * j:128 * (j + 1), k0:k0 + 128].T
    g["fc2Wp"] = w2 * 2.0

    # fc3W pack: [128(k), 2(kk), 2(pair), 33]
    w3 = np.zeros((128, 2, 2, OUT), f32)
    for kk in range(2):
        for i in range(2):
            k0 = 128 * (2 * kk + i)
            w3[:, kk, i, :] = fc3W[:, k0:k0 + 128].T
    g["W3p"] = w3 * 4.0

    g["F3rep"] = np.tile(fc3b[None, :] * 32.0, (B, 1))
    g["I32"] = np.eye(32, dtype=f32)

    oh0 = np.zeros((32, 2, B), f32)
    oh0[0, 0, :] = 1.0  # SOS onehot
    oh0[0, 1, :] = 0.0  # dur at t=0
    oh0[1, 1, :] = 1.0  # bias row
    g["oh0P"] = oh0

    cond = conditionals @ fcW.T + fcb
    cond = np.where(cond >= 0, cond, SLOPE * cond).astype(f32)
    CC2 = (cond @ fc2W.T + fc2b).astype(f32)      # [256, 512]

    per_core = []
    for ci in range(N_CORES):
        sl = slice(ci * B, (ci + 1) * B)
        pc = {}
        for l, name in ((0, "h1T0"), (1, "h2T0")):
            hc = h0[l, sl]                        # [32, 512]
            # hT[p, 32k+b] = S_H * h[b, 128k+p]
            pc[name] = np.ascontiguousarray(
                hc.reshape(B, 4, 128).transpose(2, 1, 0).reshape(128, 128)) * S_H
        for l, name in ((0, "c10"), (1, "c20")):
            cc = c0[l, sl]
            pc[name] = np.ascontiguousarray(
                cc.reshape(B, 4, 128).transpose(2, 1, 0).reshape(128, 128))
        # CC2T[b, j*128+m] = SG * CC2[b, 128j+m]
        pc["CC2T"] = np.ascontiguousarray(CC2[sl] * 8.0)
        per_core.append(pc)
    return g, per_core


# ---------------------------------------------------------------------------
# Bass program
# ---------------------------------------------------------------------------
def _region(tile_go, tile_if, c):
    """PSUM region AP for chunk c: tile_go holds g|o, tile_if holds i|f."""
    r = c % 4
    if c < 4:
        return tile_go[:, 32 * r:32 * r + 32]
    if c < 8:
        return tile_go[:, 128 + 32 * r:128 + 32 * r + 32]
    if c < 12:
        return tile_if[:, 32 * r:32 * r + 32]
    return tile_if[:, 128 + 32 * r:128 + 32 * r + 32]


def _build_program():
    import concourse.bass as bass
    import concourse.tile as tile
    from concourse import mybir, bacc

    F32 = mybir.dt.float32
    BF16 = mybir.dt.bfloat16
    FP8 = mybir.dt.float8e4
    AF = mybir.ActivationFunctionType
    ALU = mybir.AluOpType
    DR = mybir.MatmulPerfMode.DoubleRow

    nc = bacc.Bacc("TRN2", target_bir_lowering=False, debug=False)

    def din(name, shape):
        dt = FP8 if name in _FP8_NAMES else (BF16 if name in _BF16_NAMES else F32)
        return nc.dram_tensor(name, list(shape), dt, kind="ExternalInput").ap()

    d = {
        "Whh1p": din("Whh1p", (128, 16, 2, 2, 128)),
        "Wih2p": din("Wih2p", (128, 16, 2, 2, 128)),
        "Whh2p": din("Whh2p", (128, 16, 2, 2, 128)),
        "fc2Wp": din("fc2Wp", (128, 4, 2, 2, 128)),
        "W3p": din("W3p", (128, 2, 2, OUT)),
        "E1q": din("E1q", (32, 2, 2048)),
        "B2T": din("B2T", (B, 2048)),
        "CC2T": din("CC2T", (B, 512)),
        "F3rep": din("F3rep", (B, OUT)),
        "I32": din("I32", (32, 32)),
        "oh0P": din("oh0P", (32, 2, B)),
        "h1T0": din("h1T0", (128, 128)),
        "h2T0": din("h2T0", (128, 128)),
        "c10": din("c10", (128, 128)),
        "c20": din("c20", (128, 128)),
    }
    out_d = nc.dram_tensor("out", [B, 64, OUT], F32, kind="ExternalOutput").ap()

    with tile.TileContext(nc) as tc:
        import contextlib
        ctx = contextlib.ExitStack()
        with ctx:
            consts = ctx.enter_context(tc.tile_pool(name="consts", bufs=1))
            state = ctx.enter_context(tc.tile_pool(name="state", bufs=1))
            work = ctx.enter_context(tc.tile_pool(name="work", bufs=2))
            hpool = ctx.enter_context(tc.tile_pool(name="hpool", bufs=2))
            ps_g1 = ctx.enter_context(tc.tile_pool(name="ps_g1", bufs=1, space="PSUM"))
            ps_g2 = ctx.enter_context(tc.tile_pool(name="ps_g2", bufs=1, space="PSUM"))
            ps_f = ctx.enter_context(tc.tile_pool(name="ps_f", bufs=1, space="PSUM"))
            ps_p3 = ctx.enter_context(tc.tile_pool(name="ps_p3", bufs=1, space="PSUM"))
            ps_fz = ctx.enter_context(tc.tile_pool(name="ps_fz", bufs=1, space="PSUM"))
            ps_fill = ctx.enter_context(tc.tile_pool(name="ps_fill", bufs=1, space="PSUM"))

            # ---- constant tiles ----
            I32 = consts.tile([32, 32], BF16)
            Whh1p = consts.tile([128, 16, 2, 2, 128], FP8)
            Wih2p = consts.tile([128, 16, 2, 2, 128], FP8)
            Whh2p = consts.tile([128, 16, 2, 2, 128], FP8)
            fc2Wp = consts.tile([128, 4, 2, 2, 128], FP8)
            W3p = consts.tile([128, 2, 2, OUT], FP8)
            E1q = consts.tile([32, 2, 2048], FP8)
            B2T = consts.tile([B, 2048], BF16)
            CC2T = consts.tile([B, 512], BF16)
            F3rep = consts.tile([B, OUT], BF16)
            oh0P = consts.tile([32, 2, B], FP8)

            c1 = state.tile([128, 128], BF16, tag="c1")
            c2 = state.tile([128, 128], BF16, tag="c2")
            h1 = hpool.tile([128, 128], FP8, tag="h1")
            h2 = hpool.tile([128, 128], FP8, tag="h2")
            ohP = state.tile([32, 2, B], FP8, tag="ohP")

            # DMAs: first-use order, spread across queues
            nc.sync.dma_start(I32[:], d["I32"])
            nc.sync.dma_start(h1[:], d["h1T0"])
            nc.sync.dma_start(c1[:], d["c10"])
            nc.sync.dma_start(oh0P[:], d["oh0P"])
            nc.sync.dma_start(E1q[:], d["E1q"])
            # Whh1p halves split across SP and Pool queues: both land by
            # ~5-7us instead of 8.7 (step 0's critical path); the ACT queue
            # carries only Wih2p so the sigmoid table load hoists early
            nc.sync.dma_start(Whh1p[:, 0:8], d["Whh1p"][:, 0:8])
            nc.gpsimd.dma_start(h2[:], d["h2T0"])
            nc.gpsimd.dma_start(c2[:], d["c20"])
            nc.gpsimd.dma_start(Whh1p[:, 8:16], d["Whh1p"][:, 8:16])
            nc.gpsimd.dma_start(B2T[:], d["B2T"])
            nc.gpsimd.dma_start(Whh2p[:], d["Whh2p"])
            nc.scalar.dma_start(Wih2p[:], d["Wih2p"])
            nc.sync.dma_start(CC2T[:], d["CC2T"])
            nc.sync.dma_start(fc2Wp[:], d["fc2Wp"])
            nc.gpsimd.dma_start(W3p[:], d["W3p"])
            nc.gpsimd.dma_start(F3rep[:], d["F3rep"])

            nc.vector.memset(ohP[:, 1, :], 0.0)
            nc.vector.memset(ohP[0:2, 1, :], 1.0)
            # early dummy sigmoid: hoists the sigmoid-era act-table load to
            # ~2.5us (otherwise it sits behind gt(0) at ~7.5us on the chain)
            dummy0 = work.tile([32, 1], F32, tag="dummy0")
            nc.scalar.activation(dummy0[:], I32[0:32, 0:1], AF.Sigmoid)

            predbuf = state.tile([B, 64, OUT], F32, tag="predbuf")
            if T_STEPS < 64:
                nc.vector.memset(predbuf[:], 0.0)

            def gate_rounds(Gg, Gi, Wp, hT, start):
                """32 DoubleRow h-rounds for one gate tensor.

                Each PSUM tile is bank-aligned (own zero region), so when
                `start` the first matmul into EACH tile opens that tile's
                accumulation group.
                """
                for c in range(16):
                    reg = _region(Gg, Gi, c)
                    for kk in range(2):
                        nc.tensor.matmul(
                            reg, Wp[:, c, kk],
                            hT[:, 64 * kk:64 * kk + 64].rearrange(
                                "p (two b) -> p two b", two=2),
                            start=(start and kk == 0 and c in (0, 8)),
                            stop=False, perf_mode=DR,
                            skip_group_check=True)

            def bias_rounds(Gg, Gi):
                """16 bf16 identity rounds adding B2; opens each tile's group."""
                for c in range(16):
                    reg = _region(Gg, Gi, c)
                    nc.tensor.matmul(reg, B2T[:, 128 * c:128 * (c + 1)], I32[:],
                                     start=(c in (0, 8)), stop=False,
                                     skip_group_check=True)

            def x_rounds(Gg, Gi, ohs):
                """16 fp8 DoubleRow E1 rounds; closes each tile."""
                for c in range(16):
                    reg = _region(Gg, Gi, c)
                    nc.tensor.matmul(reg, E1q[:, :, 128 * c:128 * (c + 1)], ohs,
                                     start=False, stop=(c in (7, 15)),
                                     perf_mode=DR, skip_group_check=True)

            def g2x_rounds(Gg, Gi, h1T):
                """32 DoubleRow Wih2 rounds, kk-major; closes each G2 tile."""
                for kk in range(2):
                    for c in range(16):
                        reg = _region(Gg, Gi, c)
                        nc.tensor.matmul(
                            reg, Wih2p[:, c, kk],
                            h1T[:, 64 * kk:64 * kk + 64].rearrange(
                                "p (two b) -> p two b", two=2),
                            start=False, stop=(c in (7, 15) and kk == 1),
                            perf_mode=DR, skip_group_check=True)

            def nonlin(layer, Gg, Gi, c_own):
                gt = work.tile([128, 128], BF16, tag=f"gt{layer}")
                nc.scalar.activation(gt[:], Gg[:, 0:128], AF.Tanh, scale=1.0 / SG)
                sif = work.tile([128, 256], BF16, tag=f"sif{layer}")
                nc.scalar.activation(sif[:], Gi[:], AF.Sigmoid, scale=1.0 / SG)
                u = work.tile([128, 128], BF16, tag=f"u{layer}")
                nc.vector.tensor_tensor(u[:], sif[:, 0:128], gt[:], ALU.mult)
                v = work.tile([128, 128], BF16, tag=f"v{layer}")
                nc.gpsimd.tensor_tensor(v[:], sif[:, 128:256], c_own[:], ALU.mult)
                nc.vector.tensor_tensor(c_own[:, 0:64], u[:, 0:64],
                                        v[:, 0:64], ALU.add)
                nc.vector.tensor_tensor(c_own[:, 64:128], u[:, 64:128],
                                        v[:, 64:128], ALU.add)
                so = work.tile([128, 128], BF16, tag=f"so{layer}")
                nc.scalar.activation(so[:], Gg[:, 128:256], AF.Sigmoid,
                                     scale=1.0 / SG)
                tct = work.tile([128, 128], BF16, tag=f"tc{layer}")
                nc.scalar.activation(tct[:], c_own[:], AF.Tanh)
                if N_FILL_T:
                    fillers_gen(N_FILL_T, tct[:, 0:32], gt[:, 0:64])
                hn = hpool.tile([128, 128], FP8, tag=f"h{layer}")
                # halves: kk-pair 0 (cols 0:64) lands first so kk-major
                # consumer matmuls start before the second half is done
                nc.vector.scalar_tensor_tensor(hn[:, 0:64], so[:, 0:64], S_H,
                                               tct[:, 0:64],
                                               op0=ALU.mult, op1=ALU.mult)
                nc.vector.scalar_tensor_tensor(hn[:, 64:128], so[:, 64:128],
                                               S_H, tct[:, 64:128],
                                               op0=ALU.mult, op1=ALU.mult)
                return hn

            def fc2_cc2(F, Fz):
                for T_ in (F, Fz):
                    for j in range(4):
                        nc.tensor.matmul(T_[:, 32 * j:32 * j + 32],
                                         CC2T[:, 128 * j:128 * (j + 1)], I32[:],
                                         start=(j == 0), stop=False,
                                         skip_group_check=True)

            def fc2_rounds(F, Fz, h2T):
                # twin PSUM targets: the relu branch (DVE) reads F while the
                # linear branch (ACT copy) reads Fz in parallel
                for T_ in (F, Fz):
                    for kk in range(2):
                        for j in range(4):
                            nc.tensor.matmul(
                                T_[:, 32 * j:32 * j + 32], fc2Wp[:, j, kk],
                                h2T[:, 64 * kk:64 * kk + 64].rearrange(
                                    "p (two b) -> p two b", two=2),
                                start=False, stop=(j == 3 and kk == 1),
                                perf_mode=DR, skip_group_check=True)

            # ---- t=0 preamble fills ----
            G1g = ps_g1.tile([128, 256], F32, tag="G1g")
            G1i = ps_g1.tile([128, 256], F32, tag="G1i")
            gate_rounds(G1g, G1i, Whh1p, h1, start=True)
            G2g = ps_g2.tile([128, 256], F32, tag="G2g")
            G2i = ps_g2.tile([128, 256], F32, tag="G2i")
            bias_rounds(G2g, G2i)
            gate_rounds(G2g, G2i, Whh2p, h2, start=False)
            F = ps_f.tile([128, 128], F32, tag="F")
            Fz = ps_fz.tile([128, 128], F32, tag="Fz")
            # PE p-state warmup
            for i in range(4):
                nc.tensor.matmul(F[0:32, 0:32], I32[:], I32[:], start=True,
                                 stop=True, skip_group_check=True)

            # p-state fillers: junk matmuls that keep the PE busy through the
            # chain's idle windows so chain matmuls are costed at the full
            # clock (the cost model's ramp tracks the last idle->busy edge).
            # Serialized via W-W deps on one PSUM tile, so at most one filler
            # ever sits ahead of real work (~27-53ns preemption delay).
            fill_t = ps_fill.tile([32, 64], F32, tag="fill")

            def fillers(n, dep_fp8_lhsT):
                for _ in range(n):
                    nc.tensor.matmul(fill_t[:], dep_fp8_lhsT,
                                     Whh1p[:, 0, 0, :, 0:64], start=True,
                                     stop=True, perf_mode=DR,
                                     skip_group_check=True)

            def fillers_gen(n, lhsT, rhs):
                for _ in range(n):
                    nc.tensor.matmul(fill_t[:, 0:64], lhsT, rhs, start=True,
                                     stop=True, skip_group_check=True)

            def fillers34(n, dep_lhsT_34):
                for _ in range(n):
                    nc.tensor.matmul(fill_t[:], dep_lhsT_34,
                                     E1q[:, :, 0:64], start=True,
                                     stop=True, perf_mode=DR,
                                     skip_group_check=True)
            fc2_cc2(F, Fz)
            p3 = ps_p3.tile([B, OUT], F32, tag="p3")
            nc.tensor.matmul(p3[:], I32[:], F3rep[:], start=True, stop=False,
                             skip_group_check=True)

            for t in range(T_STEPS):
                tb = t % 64
                ohs = oh0P if t == 0 else ohP
                # close G1
                x_rounds(G1g, G1i, ohs[:])
                fillers34(N_FILL_A, ohs[:])
                # G2 h2-rounds for THIS step: positioned after the G1x close
                # so they cannot queue ahead of it (in-order PE queue), but
                # they drain during the L1 chain window
                if t > 0:
                    gate_rounds(G2g, G2i, Whh2p, h2, start=False)
                # L1 chain
                h1 = nonlin(1, G1g, G1i, c1)
                # close G2
                g2x_rounds(G2g, G2i, h1)
                fillers(N_FILL_B, h1[:, 0:64].rearrange(
                    "p (two b) -> p two b", two=2))
                # L2 chain
                h2 = nonlin(2, G2g, G2i, c2)
                # fc2 close
                fc2_rounds(F, Fz, h2)
                # tail: leaky split into relu and linear branches
                rb = work.tile([128, 128], FP8, tag="rb")
                nc.vector.tensor_scalar(rb[:], F[:], 0.0, float(1.0 - SLOPE),
                                        op0=ALU.max, op1=ALU.mult)
                zb = work.tile([128, 128], FP8, tag="zb")
                nc.scalar.mul(zb[:], Fz[:], SLOPE)
                fillers(N_FILL_C, rb[:, 0:64].rearrange(
                    "p (two b) -> p two b", two=2))
                p3_cur, F_cur = p3, F
                for kk in range(2):
                    nc.tensor.matmul(
                        p3_cur[:],
                        rb[:, 64 * kk:64 * kk + 64].rearrange(
                            "p (two b) -> p two b", two=2),
                        W3p[:, kk], start=False, stop=False,
                        perf_mode=DR, skip_group_check=True)
                for kk in range(2):
                    nc.tensor.matmul(
                        p3_cur[:],
                        zb[:, 64 * kk:64 * kk + 64].rearrange(
                            "p (two b) -> p two b", two=2),
                        W3p[:, kk], start=False, stop=(kk == 1),
                        perf_mode=DR, skip_group_check=True)
                if t == T_STEPS - 1:
                    # ACT switches to the exp/ln table after the loop's last
                    # Tanh; hide the 1.3us load under the remaining PE work
                    dummy = work.tile([B, 1], F32, tag="dummy")
                    nc.scalar.activation(dummy[:], c2[0:32, 0:1], AF.Exp)
                # argmax feedback
                if t < T_STEPS - 1:
                    mx = work.tile([B, 8], F32, tag="mx")
                    nc.vector.max(mx[:], p3_cur[:, 0:32])
                    oh = work.tile([B, 32], FP8, tag="oh")
                    nc.vector.tensor_scalar(oh[:], p3_cur[:, 0:32],
                                            mx[:, 0:1], None, op0=ALU.is_equal)
                    if N_FILL_O:
                        fillers_gen(N_FILL_O, oh[:],
                                    oh0P[:].rearrange("p a b -> p (a b)"))
                    nc.vector.transpose(ohP[:, 0, :], oh[:])
                # pred copy (unscale by 1/32) on DVE after the argmax ops
                # (gpsimd cannot read PSUM; ACT would block next gate acts)
                nc.vector.tensor_scalar(predbuf[:, tb, :], p3_cur[:],
                                        1.0 / 32.0, None, op0=ALU.mult)
                # ---- fills for t+1 ----
                if t + 1 < T_STEPS:
                    G1g = ps_g1.tile([128, 256], F32, tag="G1g")
                    G1i = ps_g1.tile([128, 256], F32, tag="G1i")
                    gate_rounds(G1g, G1i, Whh1p, h1, start=True)
                    G2g = ps_g2.tile([128, 256], F32, tag="G2g")
                    G2i = ps_g2.tile([128, 256], F32, tag="G2i")
                    bias_rounds(G2g, G2i)
                    F = ps_f.tile([128, 128], F32, tag="F")
                    Fz = ps_fz.tile([128, 128], F32, tag="Fz")
                    fc2_cc2(F, Fz)
                    p3 = ps_p3.tile([B, OUT], F32, tag="p3")
                    nc.tensor.matmul(p3[:], I32[:], F3rep[:], start=True,
                                     stop=False, skip_group_check=True)

            # gate tile: forces postprocess exps after the loop.  Derived from
            # the final c2 state (lands right after the loop's last c-update),
            # ~1us earlier than predbuf[63] - the first exp chunk only needs
            # predbuf[0:32], which is long done.
            gate0 = work.tile([B, 1], F32, tag="gate0")
            nc.vector.tensor_scalar(gate0[:], zb[0:B, 0:1],
                                    0.0, None, op0=ALU.mult)

            # ---- postprocess ----
            e = state.tile([B, 64, OUT], F32, tag="e")
            s = work.tile([B, 64], F32, tag="s")
            for t0 in range(0, 64, 32):
                nc.scalar.activation(e[:, t0:t0 + 32, :],
                                     predbuf[:, t0:t0 + 32, :], AF.Exp,
                                     bias=gate0[:, 0:1])
                nc.vector.tensor_reduce(s[:, t0:t0 + 32],
                                        e[:, t0:t0 + 32, 0:32],
                                        mybir.AxisListType.X, ALU.add)
            lns = work.tile([B, 64], F32, tag="lns")
            nc.scalar.activation(lns[:, 0:32], s[:, 0:32], AF.Ln)
            nc.scalar.activation(lns[:, 32:64], s[:, 32:64], AF.Ln)
            outf = state.tile([B, 64, OUT], F32, tag="outf")
            sd = work.tile([B, 1], F32, tag="sd")
            nc.vector.tensor_reduce(sd[:], e[:, :, 32:33], mybir.AxisListType.XY,
                                    ALU.add)
            rsd = work.tile([B, 1], F32, tag="rsd")
            nc.vector.reciprocal(rsd[:], sd[:])
            nc.gpsimd.tensor_scalar(outf[:, :, 32:33], e[:, :, 32:33],
                                    rsd[:, 0:1], None, op0=ALU.mult)
            dqs = (nc.sync, nc.scalar, nc.gpsimd)
            for i, t0 in enumerate(range(0, 64, 8)):
                eng = nc.vector if i % 2 == 0 else nc.gpsimd
                eng.tensor_tensor(
                    outf[:, t0:t0 + 8, 0:32], predbuf[:, t0:t0 + 8, 0:32],
                    lns[:, t0:t0 + 8].broadcast_to((B, 8, 32)),
                    ALU.subtract)
                dqs[i % 3].dma_start(
                    out_d[:, t0:t0 + 8, :], outf[:, t0:t0 + 8, :])

    nc.compile()
    return nc, out_d.tensor.name


def kernel(**inputs):
    from concourse import bass_utils

    g, per_core = _prep(inputs)
    if "prog" not in _PROGRAM_CACHE:
        _PROGRAM_CACHE["prog"] = _build_program()
    nc, out_name = _PROGRAM_CACHE["prog"]

    bf16, fp8 = _bf16np(), _fp8np()

    def conv(k, v):
        a = np.asarray(v, np.float32)
        if k in _FP8_NAMES:
            return np.ascontiguousarray(a.astype(fp8))
        if k in _BF16_NAMES:
            return np.ascontiguousarray(a.astype(bf16))
        return np.ascontiguousarray(a)

    in_maps = []
    for ci in range(N_CORES):
        m = dict(g)
        m.update(per_core[ci])
        in_maps.append({k: conv(k, v) for k, v in m.items()})
    ncores = int(os.environ.get("KERNEL_CORES", str(N_CORES)))
    kwargs = {}
    if os.environ.get("KERNEL_TRACE"):
        kwargs = dict(trace=True, tmpdir=os.environ.get("KERNEL_TRACE_DIR") or None)
    res = bass_utils.run_bass_kernel_spmd(nc, in_maps[:ncores],
                                          core_ids=list(range(ncores)), **kwargs)
    global LAST_EXEC_NS
    LAST_EXEC_NS = res.exec_time_ns
    out = np.concatenate([r[out_name] for r in res.results], axis=0)
    return out.astype(np.float32)


# revision 11
# speedup vs baseline: 1.0080x; 1.0030x over previous
"""Trainium2 Bass kernel for nn_Decoder_76974403879078 — v2 (weight-stationary).

2-layer LSTM decoder, B=256, H=512, T=64 steps, argmax feedback.
Sharding: data-parallel over batch, 8 cores x 32; the sequential time loop
runs locally per core (no collectives).

Design vs the 819us baseline (3439us fp32 original):
  - Weight-stationary matmuls: weights are the PE stationary operand
    (lhsT [K=128, M=128]), h the moving operand [K=128, N=32].  The cost
    model charges out-free-size x cycles/row, so streaming 32 batch cols
    instead of 512 gate cols cuts PE stream time ~4x.
  - fp8 e4m3 DoubleRow everywhere big: gate weights, fc2W, fc3W, the E1
    x-path table, and the h states.  One DoubleRow instruction contracts
    two K=128 tiles at 0.5 cycles/row (8x less PE stream than the
    baseline's bf16 output-stationary scheme).  Host study: full-fp8
    trajectory rel err ~4e-3 vs the 2e-2 tolerance; argmax flips are
    benign near-ties (bf16 itself flips 220/16384 with rel 3.9e-4).
    Weights x8, h x4 dodge fp8 subnormals; the 1/32 unscale folds into
    the ACT gate sigmoid/tanh `scale`.
  - Gate PSUM layout per layer: tiles (g|o) and (i|f), each its own
    bank/zero-region, closed per tile so tanh(g) fires after the go-tile
    x-rounds and sigmoid(i,f) right after the if-tile's.
  - L1 x-path: E1ext table matmul with the onehot packed as K=32
    DoubleRow pairs; dur/bias rows ride pair-slot 1 at partitions 0/1
    (constant after t=0, memset once).  argmax feedback: DVE max ->
    is_equal -> 32x32 StreamTranspose writes pair-slot 0 in place.
  - Bias/const injects (B2, CC2, F3) are bf16 identity matmuls
    (lhsT=rows, rhs=I32) that also open each bank's accumulation group -
    no warm-PSUM hacks.
  - leaky(z) -> two fp8 branches on twin PSUM copies of fc2 (tile dep
    tracking serializes same-tile readers): relu*0.99 on DVE (max+mult
    tensor_scalar) || 0.01*z on ACT (copy w/ scale); fc3 = 4 DoubleRow
    matmuls sharing one W3.
  - c-update: u = sig_i*tanh_g (DVE bf16 2x), v = sig_f*c (GPSIMD,
    parallel), c' = u+v in column halves (DVE); c state in bf16.
    h' = (sig_o*4)*tanh(c') via one STT, written fp8 in column halves so
    kk-major consumer matmuls start on the first half.
  - No transposes for h anywhere: matmul outputs land directly in the
    [hidden-part, (k-slice, batch)] layout the next matmul consumes.
  - Program order tuned for the greedy ready-first scheduler: next-step
    G1/G2 h-rounds and bias/const fills sit in the step tail and drain
    into PE idle windows; G2 h2-rounds are positioned after the G1 x-close
    so they cannot queue ahead of it.
  - Postprocess tail: exps gated on the last step's zb (the ACT queue's
    final loop op) rather than predbuf[63]; output DMA'd in 8 chunks over
    three DGE queues so the last transfer is short.

Measured (CoreSim TRN2 cost model, per core): 318.8us (4.98us/step) vs
819.0us baseline (2.55x).  Backend-validated (8-core PJRT): rel err
4.17e-3 (tolerance 2e-2).  The loop is latency-bound on the per-step
dependency cycle argmax -> onehot -> L1 -> L2 -> fc2 -> fc3 -> argmax;
PE busy is only ~30%, all engines start ops as soon as data lands.
"""
import sys
import numpy as np

sys.path.insert(0, "/opt/trn_rl_repo")

import os
HIDDEN = 512
OUT = 33
T_STEPS = int(os.environ.get("KERNEL_STEPS", "64"))
B_FULL = 256
N_CORES = 8
B = B_FULL // N_CORES  # 32
SLOPE = 0.01
N_FILL_A = int(os.environ.get("N_FILL_A", "0"))
N_FILL_B = int(os.environ.get("N_FILL_B", "0"))
N_FILL_C = int(os.environ.get("N_FILL_C", "0"))
N_FILL_T = int(os.environ.get("N_FILL_T", "0"))
N_FILL_O = int(os.environ.get("N_FILL_O", "0"))
S_W = 8.0    # fp8 weight scale
S_H = 4.0    # fp8 hidden-state scale
SG = S_W * S_H  # 32: gate-psum scale

_PROGRAM_CACHE = {}
LAST_EXEC_NS = None

_BF16_NAMES = {"B2T", "CC2T", "F3rep", "I32", "c10", "c20"}
_FP8_NAMES = {"Whh1p", "Wih2p", "Whh2p", "fc2Wp", "W3p", "h1T0", "h2T0",
              "E1q", "oh0P"}

# chunk order within each gate tensor: go-tile chunks then if-tile chunks
_QORDER = ("g", "o", "i", "f")  # chunks 0..3=g, 4..7=o, 8..11=i, 12..15=f
_TBASE = {"i": 0, "f": 512, "g": 1024, "o": 1536}  # torch gate row blocks


def _bf16np():
    import ml_dtypes
    return ml_dtypes.bfloat16


def _fp8np():
    import ml_dtypes
    return ml_dtypes.float8_e4m3fn


def _chunk_rows(c):
    """Torch-row indices for chunk c (128 gate rows)."""
    q = _QORDER[c // 4]
    r = c % 4
    return np.arange(_TBASE[q] + 128 * r, _TBASE[q] + 128 * r + 128)


def _pack_gate_w(W):
    """[2048, 512] -> fp8 lhsT pack [128(k), 16(chunk), 2(kk), 2(pair), 128(M)]."""
    out = np.zeros((128, 16, 2, 2, 128), np.float32)
    for c in range(16):
        rows = _chunk_rows(c)
        for kk in range(2):
            for i in range(2):
                k0 = 128 * (2 * kk + i)
                # lhsT[p, m] = W[rows[m], k0+p]
                out[:, c, kk, i, :] = W[rows][:, k0:k0 + 128].T
    return (out * S_W)


def _prep(inputs):
    f32 = np.float32
    emb = np.asarray(inputs["emb"], f32)
    Wih = np.asarray(inputs["Wih"], f32)
    Whh = np.asarray(inputs["Whh"], f32)
    bih = np.asarray(inputs["bih"], f32)
    bhh = np.asarray(inputs["bhh"], f32)
    fcW = np.asarray(inputs["fcW"], f32)
    fcb = np.asarray(inputs["fcb"], f32)
    fc2W = np.asarray(inputs["fc2W"], f32)
    fc2b = np.asarray(inputs["fc2b"], f32)
    fc3W = np.asarray(inputs["fc3W"], f32)
    fc3b = np.asarray(inputs["fc3b"], f32)
    h0 = np.asarray(inputs["h0"], f32)
    c0 = np.asarray(inputs["c0"], f32)
    conditionals = np.asarray(inputs["conditionals"], f32)

    g = {}
    g["Whh1p"] = _pack_gate_w(Whh[0])
    g["Wih2p"] = _pack_gate_w(Wih[1])
    g["Whh2p"] = _pack_gate_w(Whh[1])

    # E1ext: x-path lookup table for layer 1 (bf16, scaled by SG)
    # col c*128+m -> torch gate row _chunk_rows(c)[m]
    colrows = np.concatenate([_chunk_rows(c) for c in range(16)])  # [2048]
    Wih1r = Wih[0][colrows]                       # [2048, 512]
    e1 = np.zeros((34, 2048), f32)
    e1[:32] = emb @ Wih1r[:, :511].T
    e1[32] = Wih1r[:, 511]
    e1[33] = (bih[0] + bhh[0])[colrows]
    # fp8 DoubleRow pack: slot 0 = class rows, slot 1 = dur/bias rows at
    # partitions 0/1 (matching ohP's constant slot-1 layout)
    e1q = np.zeros((32, 2, 2048), f32)
    e1q[:, 0, :] = e1[:32] * SG
    e1q[0, 1, :] = e1[32] * SG
    e1q[1, 1, :] = e1[33] * SG
    g["E1q"] = e1q

    b2 = (bih[1] + bhh[1])[colrows]               # [2048] chunk-major
    g["B2T"] = np.tile((b2 * SG)[None, :], (B, 1))  # [32, 2048]

    # fc2W pack: [128(k), 4(j), 2(kk), 2(pair), 128(m)]
    w2 = np.zeros((128, 4, 2, 2, 128), f32)
    for j in range(4):
        for kk in range(2):
            for i in range(2):
                k0 = 128 * (2 * kk + i)
                w2[:, j, kk, i, :] = fc2W[128 * j:128 * (j + 1), k0:k0 + 128].T
    g["fc2Wp"] = w2 * 2.0

    # fc3W pack: [128(k), 2(kk), 2(pair), 33]
    w3 = np.zeros((128, 2, 2, OUT), f32)
    for kk in range(2):
        for i in range(2):
            k0 = 128 * (2 * kk + i)
            w3[:, kk, i, :] = fc3W[:, k0:k0 + 128].T
    g["W3p"] = w3 * 4.0

    g["F3rep"] = np.tile(fc3b[None, :] * 32.0, (B, 1))
    g["I32"] = np.eye(32, dtype=f32)

    oh0 = np.zeros((32, 2, B), f32)
    oh0[0, 0, :] = 1.0  # SOS onehot
    oh0[0, 1, :] = 0.0  # dur at t=0
    oh0[1, 1, :] = 1.0  # bias row
    g["oh0P"] = oh0

    cond = conditionals @ fcW.T + fcb
    cond = np.where(cond >= 0, cond, SLOPE * cond).astype(f32)
    CC2 = (cond @ fc2W.T + fc2b).astype(f32)      # [256, 512]

    per_core = []
    for ci in range(N_CORES):
        sl = slice(ci * B, (ci + 1) * B)
        pc = {}
        for l, name in ((0, "h1T0"), (1, "h2T0")):
            hc = h0[l, sl]                        # [32, 512]
            # hT[p, 32k+b] = S_H * h[b, 128k+p]
            pc[name] = np.ascontiguousarray(
                hc.reshape(B, 4, 128).transpose(2, 1, 0).reshape(128, 128)) * S_H
        for l, name in ((0, "c10"), (1, "c20")):
            cc = c0[l, sl]
            pc[name] = np.ascontiguousarray(
                cc.reshape(B, 4, 128).transpose(2, 1, 0).reshape(128, 128))
        # CC2T[b, j*128+m] = SG * CC2[b, 128j+m]
        pc["CC2T"] = np.ascontiguousarray(CC2[sl] * 8.0)
        per_core.append(pc)
    return g, per_core


# ---------------------------------------------------------------------------
# Bass program
# ---------------------------------------------------------------------------
def _region(tile_go, tile_if, c):
    """PSUM region AP for chunk c: tile_go holds g|o, tile_if holds i|f."""
    r = c % 4
    if c < 4:
        return tile_go[:, 32 * r:32 * r + 32]
    if c < 8:
        return tile_go[:, 128 + 32 * r:128 + 32 * r + 32]
    if c < 12:
        return tile_if[:, 32 * r:32 * r + 32]
    return tile_if[:, 128 + 32 * r:128 + 32 * r + 32]


def _build_program():
    import concourse.bass as bass
    import concourse.tile as tile
    from concourse import mybir, bacc

    F32 = mybir.dt.float32
    BF16 = mybir.dt.bfloat16
    FP8 = mybir.dt.float8e4
    AF = mybir.ActivationFunctionType
    ALU = mybir.AluOpType
    DR = mybir.MatmulPerfMode.DoubleRow

    nc = bacc.Bacc("TRN2", target_bir_lowering=False, debug=False)

    def din(name, shape):
        dt = FP8 if name in _FP8_NAMES else (BF16 if name in _BF16_NAMES else F32)
        return nc.dram_tensor(name, list(shape), dt, kind="ExternalInput").ap()

    d = {
        "Whh1p": din("Whh1p", (128, 16, 2, 2, 128)),
        "Wih2p": din("Wih2p", (128, 16, 2, 2, 128)),
        "Whh2p": din("Whh2p", (128, 16, 2, 2, 128)),
        "fc2Wp": din("fc2Wp", (128, 4, 2, 2, 128)),
        "W3p": din("W3p", (128, 2, 2, OUT)),
        "E1q": din("E1q", (32, 2, 2048)),
        "B2T": din("B2T", (B, 2048)),
        "CC2T": din("CC2T", (B, 512)),
        "F3rep": din("F3rep", (B, OUT)),
        "I32": din("I32", (32, 32)),
        "oh0P": din("oh0P", (32, 2, B)),
        "h1T0": din("h1T0", (128, 128)),
        "h2T0": din("h2T0", (128, 128)),
        "c10": din("c10", (128, 128)),
        "c20": din("c20", (128, 128)),
    }
    out_d = nc.dram_tensor("out", [B, 64, OUT], F32, kind="ExternalOutput").ap()

    with tile.TileContext(nc) as tc:
        import contextlib
        ctx = contextlib.ExitStack()
        with ctx:
            consts = ctx.enter_context(tc.tile_pool(name="consts", bufs=1))
            state = ctx.enter_context(tc.tile_pool(name="state", bufs=1))
            work = ctx.enter_context(tc.tile_pool(name="work", bufs=2))
            hpool = ctx.enter_context(tc.tile_pool(name="hpool", bufs=2))
            ps_g1 = ctx.enter_context(tc.tile_pool(name="ps_g1", bufs=1, space="PSUM"))
            ps_g2 = ctx.enter_context(tc.tile_pool(name="ps_g2", bufs=1, space="PSUM"))
            ps_f = ctx.enter_context(tc.tile_pool(name="ps_f", bufs=1, space="PSUM"))
            ps_p3 = ctx.enter_context(tc.tile_pool(name="ps_p3", bufs=1, space="PSUM"))
            ps_fz = ctx.enter_context(tc.tile_pool(name="ps_fz", bufs=1, space="PSUM"))
            ps_fill = ctx.enter_context(tc.tile_pool(name="ps_fill", bufs=1, space="PSUM"))

            # ---- constant tiles ----
            I32 = consts.tile([32, 32], BF16)
            Whh1p = consts.tile([128, 16, 2, 2, 128], FP8)
            Wih2p = consts.tile([128, 16, 2, 2, 128], FP8)
            Whh2p = consts.tile([128, 16, 2, 2, 128], FP8)
            fc2Wp = consts.tile([128, 4, 2, 2, 128], FP8)
            W3p = consts.tile([128, 2, 2, OUT], FP8)
            E1q = consts.tile([32, 2, 2048], FP8)
            B2T = consts.tile([B, 2048], BF16)
            CC2T = consts.tile([B, 512], BF16)
            F3rep = consts.tile([B, OUT], BF16)
            oh0P = consts.tile([32, 2, B], FP8)

            c1 = state.tile([128, 128], BF16, tag="c1")
            c2 = state.tile([128, 128], BF16, tag="c2")
            h1 = hpool.tile([128, 128], FP8, tag="h1")
            h2 = hpool.tile([128, 128], FP8, tag="h2")
            ohP = state.tile([32, 2, B], FP8, tag="ohP")

            # DMAs: first-use order, spread across queues
            # E1q gates every G1 x-round, so it issues second; Whh1p halves
            # split across SP and Pool queues land by ~5-6us instead of 8.7;
            # the ACT queue carries only Wih2p so the sigmoid table load
            # hoists early
            nc.sync.dma_start(I32[:], d["I32"])
            nc.sync.dma_start(E1q[:], d["E1q"])
            nc.sync.dma_start(h1[:], d["h1T0"])
            nc.sync.dma_start(c1[:], d["c10"])
            nc.sync.dma_start(oh0P[:], d["oh0P"])
            nc.sync.dma_start(Whh1p[:, 0:8], d["Whh1p"][:, 0:8])
            nc.sync.dma_start(Whh2p[:, 0:8], d["Whh2p"][:, 0:8])
            nc.gpsimd.dma_start(h2[:], d["h2T0"])
            nc.gpsimd.dma_start(c2[:], d["c20"])
            nc.gpsimd.dma_start(Whh1p[:, 8:16], d["Whh1p"][:, 8:16])
            nc.gpsimd.dma_start(B2T[:], d["B2T"])
            nc.gpsimd.dma_start(Whh2p[:, 8:16], d["Whh2p"][:, 8:16])
            nc.scalar.dma_start(Wih2p[:], d["Wih2p"])
            nc.scalar.dma_start(fc2Wp[:], d["fc2Wp"])
            nc.sync.dma_start(CC2T[:], d["CC2T"])
            nc.gpsimd.dma_start(W3p[:], d["W3p"])
            nc.gpsimd.dma_start(F3rep[:], d["F3rep"])

            nc.vector.memset(ohP[:, 1, :], 0.0)
            nc.vector.memset(ohP[0:2, 1, :], 1.0)
            # early dummy sigmoid: hoists the sigmoid-era act-table load to
            # ~2.5us (otherwise it sits behind gt(0) at ~7.5us on the chain)
            dummy0 = work.tile([32, 1], F32, tag="dummy0")
            nc.scalar.activation(dummy0[:], I32[0:32, 0:1], AF.Sigmoid)

            predbuf = state.tile([B, 64, OUT], F32, tag="predbuf")
            if T_STEPS < 64:
                nc.vector.memset(predbuf[:], 0.0)

            def gate_rounds(Gg, Gi, Wp, hT, start):
                """32 DoubleRow h-rounds for one gate tensor.

                Each PSUM tile is bank-aligned (own zero region), so when
                `start` the first matmul into EACH tile opens that tile's
                accumulation group.
                """
                for c in range(16):
                    reg = _region(Gg, Gi, c)
                    for kk in range(2):
                        nc.tensor.matmul(
                            reg, Wp[:, c, kk],
                            hT[:, 64 * kk:64 * kk + 64].rearrange(
                                "p (two b) -> p two b", two=2),
                            start=(start and kk == 0 and c in (0, 8)),
                            stop=False, perf_mode=DR,
                            skip_group_check=True)

            def bias_rounds(Gg, Gi):
                """16 bf16 identity rounds adding B2; opens each tile's group."""
                for c in range(16):
                    reg = _region(Gg, Gi, c)
                    nc.tensor.matmul(reg, B2T[:, 128 * c:128 * (c + 1)], I32[:],
                                     start=(c in (0, 8)), stop=False,
                                     skip_group_check=True)

            def x_rounds(Gg, Gi, ohs):
                """16 fp8 DoubleRow E1 rounds; closes each tile."""
                for c in range(16):
                    reg = _region(Gg, Gi, c)
                    nc.tensor.matmul(reg, E1q[:, :, 128 * c:128 * (c + 1)], ohs,
                                     start=False, stop=(c in (7, 15)),
                                     perf_mode=DR, skip_group_check=True)

            def g2x_rounds(Gg, Gi, h1T):
                """32 DoubleRow Wih2 rounds, kk-major; closes each G2 tile."""
                for kk in range(2):
                    for c in range(16):
                        reg = _region(Gg, Gi, c)
                        nc.tensor.matmul(
                            reg, Wih2p[:, c, kk],
                            h1T[:, 64 * kk:64 * kk + 64].rearrange(
                                "p (two b) -> p two b", two=2),
                            start=False, stop=(c in (7, 15) and kk == 1),
                            perf_mode=DR, skip_group_check=True)

            def nonlin(layer, Gg, Gi, c_own):
                gt = work.tile([128, 128], BF16, tag=f"gt{layer}")
                nc.scalar.activation(gt[:], Gg[:, 0:128], AF.Tanh, scale=1.0 / SG)
                sif = work.tile([128, 256], BF16, tag=f"sif{layer}")
                nc.scalar.activation(sif[:], Gi[:], AF.Sigmoid, scale=1.0 / SG)
                u = work.tile([128, 128], BF16, tag=f"u{layer}")
                nc.vector.tensor_tensor(u[:], sif[:, 0:128], gt[:], ALU.mult)
                v = work.tile([128, 128], BF16, tag=f"v{layer}")
                nc.gpsimd.tensor_tensor(v[:], sif[:, 128:256], c_own[:], ALU.mult)
                nc.vector.tensor_tensor(c_own[:, 0:64], u[:, 0:64],
                                        v[:, 0:64], ALU.add)
                nc.vector.tensor_tensor(c_own[:, 64:128], u[:, 64:128],
                                        v[:, 64:128], ALU.add)
                so = work.tile([128, 128], BF16, tag=f"so{layer}")
                nc.scalar.activation(so[:], Gg[:, 128:256], AF.Sigmoid,
                                     scale=1.0 / SG)
                tct = work.tile([128, 128], BF16, tag=f"tc{layer}")
                nc.scalar.activation(tct[:], c_own[:], AF.Tanh)
                if N_FILL_T:
                    fillers_gen(N_FILL_T, tct[:, 0:32], gt[:, 0:64])
                hn = hpool.tile([128, 128], FP8, tag=f"h{layer}")
                # halves: kk-pair 0 (cols 0:64) lands first so kk-major
                # consumer matmuls start before the second half is done
                nc.vector.scalar_tensor_tensor(hn[:, 0:64], so[:, 0:64], S_H,
                                               tct[:, 0:64],
                                               op0=ALU.mult, op1=ALU.mult)
                nc.vector.scalar_tensor_tensor(hn[:, 64:128], so[:, 64:128],
                                               S_H, tct[:, 64:128],
                                               op0=ALU.mult, op1=ALU.mult)
                return hn

            def fc2_cc2(F, Fz):
                for T_ in (F, Fz):
                    for j in range(4):
                        nc.tensor.matmul(T_[:, 32 * j:32 * j + 32],
                                         CC2T[:, 128 * j:128 * (j + 1)], I32[:],
                                         start=(j == 0), stop=False,
                                         skip_group_check=True)

            def fc2_rounds(F, Fz, h2T):
                # twin PSUM targets: the relu branch (DVE) reads F while the
                # linear branch (ACT copy) reads Fz in parallel
                for T_ in (F, Fz):
                    for kk in range(2):
                        for j in range(4):
                            nc.tensor.matmul(
                                T_[:, 32 * j:32 * j + 32], fc2Wp[:, j, kk],
                                h2T[:, 64 * kk:64 * kk + 64].rearrange(
                                    "p (two b) -> p two b", two=2),
                                start=False, stop=(j == 3 and kk == 1),
                                perf_mode=DR, skip_group_check=True)

            # ---- t=0 preamble fills ----
            G1g = ps_g1.tile([128, 256], F32, tag="G1g")
            G1i = ps_g1.tile([128, 256], F32, tag="G1i")
            gate_rounds(G1g, G1i, Whh1p, h1, start=True)
            G2g = ps_g2.tile([128, 256], F32, tag="G2g")
            G2i = ps_g2.tile([128, 256], F32, tag="G2i")
            bias_rounds(G2g, G2i)
            gate_rounds(G2g, G2i, Whh2p, h2, start=False)
            F = ps_f.tile([128, 128], F32, tag="F")
            Fz = ps_fz.tile([128, 128], F32, tag="Fz")
            # PE p-state warmup
            for i in range(4):
                nc.tensor.matmul(F[0:32, 0:32], I32[:], I32[:], start=True,
                                 stop=True, skip_group_check=True)

            # p-state fillers: junk matmuls that keep the PE busy through the
            # chain's idle windows so chain matmuls are costed at the full
            # clock (the cost model's ramp tracks the last idle->busy edge).
            # Serialized via W-W deps on one PSUM tile, so at most one filler
            # ever sits ahead of real work (~27-53ns preemption delay).
            fill_t = ps_fill.tile([32, 64], F32, tag="fill")

            def fillers(n, dep_fp8_lhsT):
                for _ in range(n):
                    nc.tensor.matmul(fill_t[:], dep_fp8_lhsT,
                                     Whh1p[:, 0, 0, :, 0:64], start=True,
                                     stop=True, perf_mode=DR,
                                     skip_group_check=True)

            def fillers_gen(n, lhsT, rhs):
                for _ in range(n):
                    nc.tensor.matmul(fill_t[:, 0:64], lhsT, rhs, start=True,
                                     stop=True, skip_group_check=True)

            def fillers34(n, dep_lhsT_34):
                for _ in range(n):
                    nc.tensor.matmul(fill_t[:], dep_lhsT_34,
                                     E1q[:, :, 0:64], start=True,
                                     stop=True, perf_mode=DR,
                                     skip_group_check=True)
            fc2_cc2(F, Fz)
            p3 = ps_p3.tile([B, OUT], F32, tag="p3")
            nc.tensor.matmul(p3[:], I32[:], F3rep[:], start=True, stop=False,
                             skip_group_check=True)

            for t in range(T_STEPS):
                tb = t % 64
                ohs = oh0P if t == 0 else ohP
                # close G1
                x_rounds(G1g, G1i, ohs[:])
                fillers34(N_FILL_A, ohs[:])
                # G2 h2-rounds for THIS step: positioned after the G1x close
                # so they cannot queue ahead of it (in-order PE queue), but
                # they drain during the L1 chain window
                if t > 0:
                    gate_rounds(G2g, G2i, Whh2p, h2, start=False)
                # L1 chain
                h1 = nonlin(1, G1g, G1i, c1)
                # close G2
                g2x_rounds(G2g, G2i, h1)
                fillers(N_FILL_B, h1[:, 0:64].rearrange(
                    "p (two b) -> p two b", two=2))
                # L2 chain
                h2 = nonlin(2, G2g, G2i, c2)
                # fc2 close
                fc2_rounds(F, Fz, h2)
                # tail: leaky split into relu and linear branches
                rb = work.tile([128, 128], FP8, tag="rb")
                nc.vector.tensor_scalar(rb[:], F[:], 0.0, float(1.0 - SLOPE),
                                        op0=ALU.max, op1=ALU.mult)
                zb = work.tile([128, 128], FP8, tag="zb")
                nc.scalar.mul(zb[:], Fz[:], SLOPE)
                fillers(N_FILL_C, rb[:, 0:64].rearrange(
                    "p (two b) -> p two b", two=2))
                p3_cur, F_cur = p3, F
                for kk in range(2):
                    nc.tensor.matmul(
                        p3_cur[:],
                        rb[:, 64 * kk:64 * kk + 64].rearrange(
                            "p (two b) -> p two b", two=2),
                        W3p[:, kk], start=False, stop=False,
                        perf_mode=DR, skip_group_check=True)
                for kk in range(2):
                    nc.tensor.matmul(
                        p3_cur[:],
                        zb[:, 64 * kk:64 * kk + 64].rearrange(
                            "p (two b) -> p two b", two=2),
                        W3p[:, kk], start=False, stop=(kk == 1),
                        perf_mode=DR, skip_group_check=True)
                if t == T_STEPS - 1:
                    # ACT switches to the exp/ln table after the loop's last
                    # Tanh; hide the 1.3us load under the remaining PE work
                    dummy = work.tile([B, 1], F32, tag="dummy")
                    nc.scalar.activation(dummy[:], c2[0:32, 0:1], AF.Exp)
                # argmax feedback
                if t < T_STEPS - 1:
                    mx = work.tile([B, 8], F32, tag="mx")
                    nc.vector.max(mx[:], p3_cur[:, 0:32])
                    oh = work.tile([B, 32], FP8, tag="oh")
                    nc.vector.tensor_scalar(oh[:], p3_cur[:, 0:32],
                                            mx[:, 0:1], None, op0=ALU.is_equal)
                    if N_FILL_O:
                        fillers_gen(N_FILL_O, oh[:],
                                    oh0P[:].rearrange("p a b -> p (a b)"))
                    nc.vector.transpose(ohP[:, 0, :], oh[:])
                # pred copy (unscale by 1/32) on DVE after the argmax ops
                # (gpsimd cannot read PSUM; ACT would block next gate acts)
                nc.vector.tensor_scalar(predbuf[:, tb, :], p3_cur[:],
                                        1.0 / 32.0, None, op0=ALU.mult)
                # ---- fills for t+1 ----
                if t + 1 < T_STEPS:
                    G1g = ps_g1.tile([128, 256], F32, tag="G1g")
                    G1i = ps_g1.tile([128, 256], F32, tag="G1i")
                    gate_rounds(G1g, G1i, Whh1p, h1, start=True)
                    G2g = ps_g2.tile([128, 256], F32, tag="G2g")
                    G2i = ps_g2.tile([128, 256], F32, tag="G2i")
                    bias_rounds(G2g, G2i)
                    F = ps_f.tile([128, 128], F32, tag="F")
                    Fz = ps_fz.tile([128, 128], F32, tag="Fz")
                    fc2_cc2(F, Fz)
                    p3 = ps_p3.tile([B, OUT], F32, tag="p3")
                    nc.tensor.matmul(p3[:], I32[:], F3rep[:], start=True,
                                     stop=False, skip_group_check=True)

            # gate tile: forces postprocess exps after the loop.  Derived from
            # the final c2 state (lands right after the loop's last c-update),
            # ~1us earlier than predbuf[63] - the first exp chunk only needs
            # predbuf[0:32], which is long done.
            gate0 = work.tile([B, 1], F32, tag="gate0")
            nc.vector.tensor_scalar(gate0[:], zb[0:B, 0:1],
                                    0.0, None, op0=ALU.mult)

            # ---- postprocess ----
            e = state.tile([B, 64, OUT], F32, tag="e")
            s = work.tile([B, 64], F32, tag="s")
            for t0 in range(0, 64, 32):
                nc.scalar.activation(e[:, t0:t0 + 32, :],
                                     predbuf[:, t0:t0 + 32, :], AF.Exp,
                                     bias=gate0[:, 0:1])
                nc.vector.tensor_reduce(s[:, t0:t0 + 32],
                                        e[:, t0:t0 + 32, 0:32],
                                        mybir.AxisListType.X, ALU.add)
            lns = work.tile([B, 64], F32, tag="lns")
            nc.scalar.activation(lns[:, 0:32], s[:, 0:32], AF.Ln)
            nc.scalar.activation(lns[:, 32:64], s[:, 32:64], AF.Ln)
            outf = state.tile([B, 64, OUT], F32, tag="outf")
            sd = work.tile([B, 1], F32, tag="sd")
            nc.vector.tensor_reduce(sd[:], e[:, :, 32:33], mybir.AxisListType.XY,
                                    ALU.add)
            rsd = work.tile([B, 1], F32, tag="rsd")
            nc.vector.reciprocal(rsd[:], sd[:])
            nc.gpsimd.tensor_scalar(outf[:, :, 32:33], e[:, :, 32:33],
                                    rsd[:, 0:1], None, op0=ALU.mult)
            dqs = (nc.sync, nc.scalar, nc.gpsimd)
            for i, t0 in enumerate(range(0, 64, 8)):
                eng = nc.vector if i % 2 == 0 else nc.gpsimd
                eng.tensor_tensor(
                    outf[:, t0:t0 + 8, 0:32], predbuf[:, t0:t0 + 8, 0:32],
                    lns[:, t0:t0 + 8].broadcast_to((B, 8, 32)),
                    ALU.subtract)
                dqs[i % 3].dma_start(
                    out_d[:, t0:t0 + 8, :], outf[:, t0:t0 + 8, :])

    nc.compile()
    return nc, out_d.tensor.name


def kernel(**inputs):
    from concourse import bass_utils

    g, per_core = _prep(inputs)
    if "prog" not in _PROGRAM_CACHE:
        _PROGRAM_CACHE["prog"] = _build_program()
    nc, out_name = _PROGRAM_CACHE["prog"]

    bf16, fp8 = _bf16np(), _fp8np()

    def conv(k, v):
        a = np.asarray(v, np.float32)
        if k in _FP8_NAMES:
            return np.ascontiguousarray(a.astype(fp8))
        if k in _BF16_NAMES:
            return np.ascontiguousarray(a.astype(bf16))
        return np.ascontiguousarray(a)

    in_maps = []
    for ci in range(N_CORES):
        m = dict(g)
        m.update(per_core[ci])
        in_maps.append({k: conv(k, v) for k, v in m.items()})
    ncores = int(os.environ.get("KERNEL_CORES", str(N_CORES)))
    kwargs = {}
    if os.environ.get("KERNEL_TRACE"):
        kwargs = dict(trace=True, tmpdir=os.environ.get("KERNEL_TRACE_DIR") or None)
    res = bass_utils.run_bass_kernel_spmd(nc, in_maps[:ncores],
                                          core_ids=list(range(ncores)), **kwargs)
    global LAST_EXEC_NS
    LAST_EXEC_NS = res.exec_time_ns
    out = np.concatenate([r[out_name] for r in res.results], axis=0)
    return out.astype(np.float32)
